# revision 18
# baseline (speedup 1.0000x reference)
"""Trainium2 Bass kernel for nn_AttentionDecoder (Bahdanau attention + GRU greedy decoder).

Sharding: pure data parallel, B=2048 split as 256 rows per core across 8 cores.
All compute in bf16 with f32 PSUM accumulation.

v2 layout scheme (per core, BL=256):
  - Batch processed as two 128-row halves whose per-step chains are emitted
    phase-interleaved so PE work of one half overlaps ACT/DVE work of the other.
  - ep/att feature-major [a, ab, t, b] (t-major free order so the dec-proj
    broadcast add keeps unit stride for DVE 2x mode); encT built t-major by
    strided-row xbar DMA transposes so the prologue ep copies are contiguous.
  - energy + context via block-diagonal attention matmuls (enc streams through
    PE as stationaries), GRU gates feature-major (weight chunks stationary)
    with every gate bias folded into the always-1 row 37 of onehotT or ones1
    bias matmuls; h stays feature-major forever (no h transposes).
  - sigmoid computed as (1+tanh(x/2))/2 so tanh/exp/relu/copy share one ACT
    table set (no per-step ACT_TABLE_LOAD).
"""

import os
import threading
import numpy as np
import ml_dtypes

N_CORES = 8
B, T, ENC = 2048, 32, 512
DEC, ATT, EMB, NCLS, L = 256, 256, 64, 37, 10
BL = B // N_CORES  # 256 per core

_BF = ml_dtypes.bfloat16

_lock = threading.Lock()
_cache = {}


def _build():
    import concourse.bass as bass
    import concourse.tile as tile
    from concourse import bacc, mybir

    bf = mybir.dt.bfloat16
    f32 = mybir.dt.float32

    nc = bacc.Bacc("TRN2", target_bir_lowering=False, debug=False,
                   num_devices=N_CORES)

    # ---------------- DRAM parameters ----------------
    d_enc = nc.dram_tensor("enc", [BL, T, ENC], bf, kind="ExternalInput").ap()
    d_wdec = nc.dram_tensor("w_dec", [DEC, ATT], bf, kind="ExternalInput").ap()
    d_wenc = nc.dram_tensor("w_enc", [ENC, ATT], bf, kind="ExternalInput").ap()
    d_v = nc.dram_tensor("v", [ATT, 1], bf, kind="ExternalInput").ap()
    d_embWb = nc.dram_tensor("embWb", [NCLS + 1, 3 * DEC], bf, kind="ExternalInput").ap()
    d_wihc = nc.dram_tensor("w_ih_c", [ENC, 3 * DEC], bf, kind="ExternalInput").ap()
    d_whhrz = nc.dram_tensor("w_hh_rz", [DEC, 2 * DEC], bf, kind="ExternalInput").ap()
    d_whhn = nc.dram_tensor("w_hh_n", [DEC, DEC], bf, kind="ExternalInput").ap()
    d_bhhn = nc.dram_tensor("b_hh_n", [1, DEC], bf, kind="ExternalInput").ap()
    d_fc1h = nc.dram_tensor("fc1_w_h", [DEC, DEC], bf, kind="ExternalInput").ap()
    d_fc1c = nc.dram_tensor("fc1_w_c", [ENC, DEC], bf, kind="ExternalInput").ap()
    d_fc1b = nc.dram_tensor("fc1_b", [1, DEC], bf, kind="ExternalInput").ap()
    d_fc2w = nc.dram_tensor("fc2_w", [DEC, NCLS], bf, kind="ExternalInput").ap()
    d_fc2b = nc.dram_tensor("fc2_b", [1, NCLS], bf, kind="ExternalInput").ap()
    d_ihw = nc.dram_tensor("init_h_w", [ENC, DEC], bf, kind="ExternalInput").ap()
    d_ihb = nc.dram_tensor("init_h_b", [DEC, 1], f32, kind="ExternalInput").ap()
    d_out = nc.dram_tensor("out", [BL, L, NCLS], f32, kind="ExternalOutput").ap()
    KDBG = bool(int(os.environ.get("KDBG", "0")))
    if KDBG:
        d_dbg_ep = nc.dram_tensor("dbg_ep", [128, 2, T, 256], bf, kind="ExternalOutput").ap()
        d_dbg_mean = nc.dram_tensor("dbg_mean", [128, 4, BL], bf, kind="ExternalOutput").ap()
        d_dbg_h0 = nc.dram_tensor("dbg_h0", [128, 2, BL], bf, kind="ExternalOutput").ap()
        d_dbg_att = nc.dram_tensor("dbg_att", [128, 2, T, 256], bf, kind="ExternalOutput").ap()
        d_dbg_attn = nc.dram_tensor("dbg_attn", [2, 128, T], f32, kind="ExternalOutput").ap()
        d_dbg_ctx = nc.dram_tensor("dbg_ctx", [128, 4, BL], bf, kind="ExternalOutput").ap()
        d_dbg_h1 = nc.dram_tensor("dbg_h1", [128, 2, BL], bf, kind="ExternalOutput").ap()

    ident_np = np.eye(128, dtype=np.float32)
    d_ident = nc.inline_tensor(ident_np, name="identf").ap()
    d_onesrow = nc.inline_tensor(np.ones((1, BL), dtype=_BF), name="onesrow").ap()

    AluOp = mybir.AluOpType
    ActF = mybir.ActivationFunctionType

    with tile.TileContext(nc) as tc:
        with (
            tc.tile_pool(name="persist", bufs=1) as P,
            tc.tile_pool(name="wpool", bufs=1) as W,
            tc.tile_pool(name="trans", bufs=2) as TR,
            tc.tile_pool(name="small", bufs=2) as SM,
            tc.tile_pool(name="ps", bufs=1, space="PSUM") as PS,
        ):
            # ---------------- persistent SBUF tensors ----------------
            enc_bd = P.tile([128, 64, ENC], bf, tag="enc_bd")       # 64KB/part
            ep = P.tile([128, 2, T, 256], bf, tag="ep")             # enc_proj, t-major
            att = P.tile([128, 2, T, 256], bf, tag="att")           # tanh buffer
            hT = P.tile([128, 2, BL], bf, tag="hT")                 # h feature-major
            ctxT = P.tile([128, 4, BL], bf, tag="ctxT")             # ctx feature-major
            hidT = P.tile([128, 2, BL], bf, tag="hidT")             # fc1 out
            onehotT = P.tile([NCLS + 1, BL], bf, tag="onehotT")
            meanT = P.tile([128, 4, BL], bf, tag="meanT")
            out_sb = P.tile([128, 2, L, NCLS], f32, tag="out_sb")
            ones1 = P.tile([1, BL], bf, tag="ones1")

            # ---------------- weights to SBUF ----------------
            def wload(tag, shape, src, rearr=None):
                t = W.tile(shape, bf, tag=tag, name=tag)
                nc.scalar.dma_start(t[:], src if rearr is None else src.rearrange(rearr, p=128))
                return t

            w_dec = wload("w_dec", [128, 2, ATT], d_wdec, "(k p) n -> p k n")
            w_enc = wload("w_enc", [128, 4, ATT], d_wenc, "(k p) n -> p k n")
            v_sb = wload("v_sb", [128, 2, 1], d_v, "(k p) n -> p k n")
            embWb = wload("embWb", [NCLS + 1, 3 * DEC], d_embWb)
            w_ihc = wload("w_ihc", [128, 4, 3 * DEC], d_wihc, "(k p) n -> p k n")
            w_hhrz = wload("w_hhrz", [128, 2, 2 * DEC], d_whhrz, "(k p) n -> p k n")
            w_hhn = wload("w_hhn", [128, 2, DEC], d_whhn, "(k p) n -> p k n")
            bhhn = wload("bhhn", [1, DEC], d_bhhn)
            fc1h = wload("fc1h", [128, 2, DEC], d_fc1h, "(k p) n -> p k n")
            fc1c = wload("fc1c", [128, 4, DEC], d_fc1c, "(k p) n -> p k n")
            fc1b = wload("fc1b", [1, DEC], d_fc1b)
            fc2w = wload("fc2w", [128, 2, NCLS], d_fc2w, "(k p) n -> p k n")
            fc2b = wload("fc2b", [1, NCLS], d_fc2b)
            ihw = wload("ihw", [128, 4, DEC], d_ihw, "(k p) n -> p k n")
            identf = W.tile([128, 128], f32, tag="identf", name="identf")
            nc.scalar.dma_start(identf[:], d_ident)
            ihb = W.tile([128, 2, 1], f32)
            nc.scalar.dma_start(ihb[:], d_ihb.rearrange("(k p) n -> p k n", p=128))

            nc.vector.memset(ones1[:], 1.0)
            nc.vector.memset(onehotT[0:NCLS, :], 0)
            nc.vector.memset(onehotT[0:1, :], 1.0)
            nc.scalar.dma_start(onehotT[NCLS:NCLS + 1, :], d_onesrow[:])

            # ---------------- prologue ----------------
            # encT built T-MAJOR (cols = (t, b)) by strided-row xbar DMA
            # transposes split across the two HWDGE queues (sync + scalar);
            # ep matmul outputs are then contiguous t-major slabs.
            for bs in range(4):
                nc.scalar.dma_start(
                    enc_bd[32 * bs:32 * bs + 32, :, :],
                    d_enc[bs::4].rearrange("g t e -> t g e"),
                )
            d_enc2d = d_enc.rearrange("b t e -> (b t) e")
            for bth in range(2):
                hs2 = 128 * bth
                encTh = TR.tile([128, 4, 4096], bf, tag="encTh", bufs=1)
                for eb in range(4):
                    eng = nc.sync
                    eng.dma_start_transpose(
                        encTh[:, eb, :],
                        d_enc2d[4096 * bth:4096 * bth + 4096,
                                128 * eb:128 * eb + 128])
                # mean over t (inner reduce); split DVE / Pool
                with nc.allow_low_precision(reason="t-sum of O(1) bf16 values for h0 mean"):
                    for eb in range(4):
                        eng = nc.vector
                        eng.tensor_reduce(
                            meanT[:, eb, hs2:hs2 + 128],
                            encTh[:, eb, :].rearrange("p (b t) -> p b t", t=T),
                            axis=mybir.AxisListType.X, op=AluOp.add)
                # ep matmuls with t-major strided moving operand: each c-slab
                # covers t 4c..4c+3 for all 128 b, so pp copies are contiguous
                for ab in range(2):
                    encT_tm = encTh.rearrange("p k (b t) -> p k t b", t=T)
                    for c in range(8):
                        pp = PS.tile([128, 512], f32, tag="mix", bufs=2)
                        for eb in range(4):
                            nc.tensor.matmul(
                                pp[:],
                                w_enc[:, eb, 128 * ab:128 * ab + 128],
                                encT_tm[:, eb, 4 * c:4 * c + 4, :],
                                start=(eb == 0), stop=(eb == 3),
                            )
                        dst = ep[:, ab, 4 * c:4 * c + 4, hs2:hs2 + 128]
                        src = pp[:].rearrange("p (t b) -> p t b", b=128)
                        if c % 2 == 0:
                            nc.vector.tensor_copy(dst, src)
                        else:
                            nc.scalar.copy(dst, src)

            # h0 = tanh(mean_enc @ (init_h_W/32) + b)   (feature-major)
            for db in range(2):
                hp = PS.tile([128, BL], f32, tag="cp", bufs=2)
                for eb in range(4):
                    nc.tensor.matmul(hp[:], ihw[:, eb, 128 * db:128 * db + 128],
                                     meanT[:, eb, :], start=(eb == 0), stop=(eb == 3))
                nc.scalar.activation(hT[:, db, :], hp[:], ActF.Tanh, bias=ihb[:, db, :])

            if KDBG:
                nc.sync.dma_start(d_dbg_ep, ep[:])
                nc.sync.dma_start(d_dbg_mean, meanT[:])
                nc.sync.dma_start(d_dbg_h0, hT[:])

            # ---------------- decode loop ----------------
            for step in range(L):
                # dec_proj feature-major: dp[ab] = W_dec[:,ab]^T h   [a, b]
                dp = PS.tile([128, 2, BL], f32, tag="dp", bufs=1)
                for ab in range(2):
                    for db in range(2):
                        nc.tensor.matmul(dp[:, ab, :],
                                         w_dec[:, db, 128 * ab:128 * ab + 128],
                                         hT[:, db, :], start=(db == 0), stop=(db == 1))
                decT = SM.tile([128, 2, BL], bf, tag="decT")
                for ab in range(2):
                    nc.vector.tensor_copy(decT[:, ab, :], dp[:, ab, :])

                # s = ep + dec (broadcast over t), tanh in place; emitted in
                # (half, ab, t-half) waves so ACT starts while DVE continues
                for half in range(2):
                    hs, he = 128 * half, 128 * half + 128
                    for ab in range(2):
                        for th in range(2):
                            ts = 16 * th
                            bcast = decT[:, ab, hs:he].rearrange(
                                "p (o b) -> p o b", o=1).broadcast_to([128, 16, 128])
                            nc.vector.tensor_tensor(
                                att[:, ab, ts:ts + 16, hs:he],
                                ep[:, ab, ts:ts + 16, hs:he], bcast, op=AluOp.add)
                            nc.scalar.activation(att[:, ab, ts:ts + 16, hs:he],
                                                 att[:, ab, ts:ts + 16, hs:he],
                                                 ActF.Tanh)

                mixes = [None, None]
                # energy (batch-major [b, t]) via flipped vdot
                for half in range(2):
                    hs, he = 128 * half, 128 * half + 128
                    mix = PS.tile([128, 512], f32, tag="mix", bufs=2)
                    mixes[half] = mix
                    for t in range(T):
                        for ab in range(2):
                            nc.tensor.matmul(mix[:, t:t + 1],
                                             att[:, ab, t, hs:he],
                                             v_sb[:, ab, :],
                                             start=(ab == 0), stop=(ab == 1))

                # softmax + attn transpose + blockdiag build
                bds = [None, None]
                for half in range(2):
                    hs, he = 128 * half, 128 * half + 128
                    mix = mixes[half]
                    expB = SM.tile([128, T], f32, tag="expB")
                    zc = SM.tile([128, 1], f32, tag="zc")
                    nc.scalar.activation(expB[:], mix[:, 0:T], ActF.Exp,
                                         accum_out=zc[:])
                    rz = SM.tile([128, 1], f32, tag="rz")
                    nc.vector.reciprocal(rz[:], zc[:])
                    attnB = SM.tile([128, T], f32, tag="attnB")
                    nc.vector.tensor_scalar(attnB[:], expB[:], rz[:], None,
                                            op0=AluOp.mult)
                    nc.tensor.transpose(mix[0:T, 128:256], attnB[:], identf[:])
                    if KDBG and step == 0:
                        nc.sync.dma_start(d_dbg_attn[half], attnB[:])
                    attn_bd = SM.tile([128, 32, 4], bf, tag="attn_bd")
                    bds[half] = attn_bd
                    nc.vector.memset(attn_bd[:], 0)
                    for bs in range(4):
                        nc.vector.tensor_copy(
                            attn_bd[32 * bs:32 * bs + 32, :, bs],
                            mix[0:T, 128 + bs:256:4],
                        )

                # context feature-major (flipped blockdiag)
                for half in range(2):
                    hs, he = 128 * half, 128 * half + 128
                    attn_bd = bds[half]
                    cpt = PS.tile([128, 4, 128], f32, tag="cp", bufs=2)
                    for eb in range(4):
                        for gr in range(32):
                            g = 32 * half + gr
                            nc.tensor.matmul(cpt[:, eb, 4 * gr:4 * gr + 4],
                                             enc_bd[:, g, 128 * eb:128 * eb + 128],
                                             attn_bd[:, gr, :], start=True, stop=True)
                    for eb in range(4):
                        if eb % 2 == 0:
                            nc.vector.tensor_copy(ctxT[:, eb, hs:he], cpt[:, eb, :])
                        else:
                            nc.scalar.copy(ctxT[:, eb, hs:he], cpt[:, eb, :])

                if KDBG and step == 0:
                    nc.sync.dma_start(d_dbg_att, att[:])
                    nc.sync.dma_start(d_dbg_ctx, ctxT[:])

                # GRU gates feature-major (weight chunks stationary)
                gps = [None, None]
                for half in range(2):
                    hs, he = 128 * half, 128 * half + 128
                    oh = onehotT[:, hs:he]
                    rzp = PS.tile([128, 4, 128], f32, tag="gp", bufs=3)
                    ngp = PS.tile([128, 4, 128], f32, tag="gp", bufs=3)
                    gps[half] = (rzp, ngp)
                    for fc in range(4):
                        fs = 128 * fc
                        nc.tensor.matmul(rzp[:, fc, :], embWb[:, fs:fs + 128], oh,
                                         start=True, stop=False)
                        for eb in range(4):
                            nc.tensor.matmul(rzp[:, fc, :],
                                             w_ihc[:, eb, fs:fs + 128],
                                             ctxT[:, eb, hs:he],
                                             start=False, stop=False)
                        for db in range(2):
                            nc.tensor.matmul(rzp[:, fc, :],
                                             w_hhrz[:, db, fs:fs + 128],
                                             hT[:, db, hs:he],
                                             start=False, stop=(db == 1))
                    for fc in range(2):
                        fs = 128 * fc
                        # n-pre: i_n (emb row bias included) + ctx + 0.5*hn + 0.5*bhhn
                        nc.tensor.matmul(ngp[:, fc, :],
                                         embWb[:, 512 + fs:512 + fs + 128], oh,
                                         start=True, stop=False)
                        for eb in range(4):
                            nc.tensor.matmul(ngp[:, fc, :],
                                             w_ihc[:, eb, 512 + fs:512 + fs + 128],
                                             ctxT[:, eb, hs:he],
                                             start=False, stop=False)
                        nc.tensor.matmul(ngp[:, fc, :], bhhn[:, fs:fs + 128],
                                         ones1[:, hs:he], start=False, stop=False)
                        for db in range(2):
                            nc.tensor.matmul(ngp[:, fc, :],
                                             w_hhn[:, db, fs:fs + 128],
                                             hT[:, db, hs:he],
                                             start=False, stop=(db == 1))
                        # ghn = 0.5*hn + 0.5*bhhn
                        nc.tensor.matmul(ngp[:, 2 + fc, :], bhhn[:, fs:fs + 128],
                                         ones1[:, hs:he], start=True, stop=False)
                        for db in range(2):
                            nc.tensor.matmul(ngp[:, 2 + fc, :],
                                             w_hhn[:, db, fs:fs + 128],
                                             hT[:, db, hs:he],
                                             start=False, stop=(db == 1))

                # GRU elementwise, feature-major; h updated in place
                for half in range(2):
                    hs, he = 128 * half, 128 * half + 128
                    rzp, ngp = gps[half]
                    tr = SM.tile([128, 2, 128], bf, tag="tr")
                    nc.scalar.activation(tr[:], rzp[:, 0:2, :], ActF.Tanh, scale=0.5)
                    tz = SM.tile([128, 2, 128], bf, tag="tz")
                    nc.scalar.activation(tz[:], rzp[:, 2:4, :], ActF.Tanh, scale=0.5)
                    rhn = SM.tile([128, 2, 128], bf, tag="rhn")
                    nc.vector.tensor_tensor(rhn[:], tr[:], ngp[:, 2:4, :],
                                            op=AluOp.mult)
                    npre = SM.tile([128, 2, 128], bf, tag="npre")
                    nc.vector.tensor_tensor(npre[:], ngp[:, 0:2, :], rhn[:],
                                            op=AluOp.add)
                    nn_ = SM.tile([128, 2, 128], bf, tag="nn_")
                    nc.scalar.activation(nn_[:], npre[:], ActF.Tanh)
                    t1 = SM.tile([128, 2, 128], bf, tag="t1")
                    nc.vector.tensor_tensor(t1[:], hT[:, :, hs:he], nn_[:],
                                            op=AluOp.subtract)
                    t2 = SM.tile([128, 2, 128], bf, tag="t2")
                    nc.vector.scalar_tensor_tensor(t2[:], tz[:], 1.0, t1[:],
                                                   op0=AluOp.add, op1=AluOp.mult)
                    nc.vector.scalar_tensor_tensor(hT[:, :, hs:he], t2[:], 0.5,
                                                   nn_[:],
                                                   op0=AluOp.mult, op1=AluOp.add)

                if KDBG and step == 0:
                    nc.sync.dma_start(d_dbg_h1, hT[:])

                # fc1 (feature-major) + fc2 (batch-major) + greedy argmax
                for half in range(2):
                    hs, he = 128 * half, 128 * half + 128
                    fp = PS.tile([128, 2, 128], f32, tag="gp", bufs=3)
                    for dc in range(2):
                        ds = 128 * dc
                        nc.tensor.matmul(fp[:, dc, :], fc1b[:, ds:ds + 128],
                                         ones1[:, hs:he], start=True, stop=False)
                        for db in range(2):
                            nc.tensor.matmul(fp[:, dc, :],
                                             fc1h[:, db, ds:ds + 128],
                                             hT[:, db, hs:he],
                                             start=False, stop=False)
                        for eb in range(4):
                            nc.tensor.matmul(fp[:, dc, :],
                                             fc1c[:, eb, ds:ds + 128],
                                             ctxT[:, eb, hs:he],
                                             start=False, stop=(eb == 3))
                    nc.scalar.activation(hidT[:, :, hs:he], fp[:], ActF.Relu)

                    mix = mixes[half]
                    nc.tensor.matmul(mix[:, 256:256 + NCLS], ones1[:, hs:he],
                                     fc2b[:], start=True, stop=False)
                    for db in range(2):
                        nc.tensor.matmul(mix[:, 256:256 + NCLS],
                                         hidT[:, db, hs:he], fc2w[:, db, :],
                                         start=False, stop=(db == 1))
                    nc.scalar.copy(out_sb[:, half, step, :], mix[:, 256:256 + NCLS])
                    if step < L - 1:
                        mx = SM.tile([128, 1], f32, tag="mx")
                        nc.vector.tensor_reduce(mx[:], mix[:, 256:256 + NCLS],
                                                axis=mybir.AxisListType.X,
                                                op=AluOp.max)
                        ohB = SM.tile([128, NCLS], f32, tag="ohB")
                        nc.vector.tensor_tensor(
                            ohB[:], mix[:, 256:256 + NCLS],
                            mx[:].broadcast_to([128, NCLS]), op=AluOp.is_equal)
                        nc.tensor.transpose(mix[0:NCLS, 320:448], ohB[:], identf[:])
                        nc.vector.tensor_copy(onehotT[0:NCLS, hs:he],
                                              mix[0:NCLS, 320:448])

            # ---------------- output DMA ----------------
            for half in range(2):
                nc.sync.dma_start(
                    d_out[128 * half:128 * half + 128],
                    out_sb[:, half, :, :],
                )

    nc.compile()
    return nc


def _get_nc():
    with _lock:
        if "nc" not in _cache:
            _cache["nc"] = _build()
        return _cache["nc"]


def kernel(**inputs):
    nc = _get_nc()
    from concourse.bass_utils import run_bass_kernel_spmd

    enc = np.ascontiguousarray(inputs["encoder_outputs"], dtype=np.float32)
    emb = inputs["emb"].astype(np.float32)
    W_enc = inputs["W_enc"].astype(np.float32)
    W_dec = inputs["W_dec"].astype(np.float32)
    v = inputs["v"].astype(np.float32)
    init_h_W = inputs["init_h_W"].astype(np.float32)
    init_h_b = inputs["init_h_b"].astype(np.float32)
    W_ih = inputs["W_ih"].astype(np.float32)
    b_ih = inputs["b_ih"].astype(np.float32)
    W_hh = inputs["W_hh"].astype(np.float32)
    b_hh = inputs["b_hh"].astype(np.float32)
    fc1_W = inputs["fc1_W"].astype(np.float32)
    fc1_b = inputs["fc1_b"].astype(np.float32)
    fc2_W = inputs["fc2_W"].astype(np.float32)
    fc2_b = inputs["fc2_b"].astype(np.float32)

    # host precompute: embedding projected through W_ih (emb part) + rz biases;
    # W_hh_n/b_hh_n halved for the tanh-form sigmoid r-gate; init_h_W absorbs
    # the 1/T mean divisor.
    bias_row = np.concatenate([(b_ih + b_hh)[:2 * DEC], b_ih[2 * DEC:]])
    embWb = np.concatenate([emb @ W_ih[:EMB], bias_row[None, :]], axis=0)

    bfc = lambda a: np.ascontiguousarray(a, dtype=_BF)
    shared = {
        "w_dec": bfc(W_dec),
        "w_enc": bfc(W_enc),
        "v": bfc(v.reshape(ATT, 1)),
        "embWb": bfc(embWb),
        "w_ih_c": bfc(W_ih[EMB:]),
        "w_hh_rz": bfc(W_hh[:, :2 * DEC]),
        "w_hh_n": bfc(0.5 * W_hh[:, 2 * DEC:]),
        "b_hh_n": bfc(0.5 * b_hh[2 * DEC:].reshape(1, DEC)),
        "fc1_w_h": bfc(fc1_W[:DEC]),
        "fc1_w_c": bfc(fc1_W[DEC:]),
        "fc1_b": bfc(fc1_b.reshape(1, DEC)),
        "fc2_w": bfc(fc2_W),
        "fc2_b": bfc(fc2_b.reshape(1, NCLS)),
        "init_h_w": bfc(init_h_W / T),
        "init_h_b": np.ascontiguousarray(init_h_b.reshape(DEC, 1), dtype=np.float32),
    }
    enc_bf = enc.astype(_BF)
    in_maps = []
    for i in range(N_CORES):
        m = dict(shared)
        m["enc"] = np.ascontiguousarray(enc_bf[i * BL:(i + 1) * BL])
        in_maps.append(m)

    res = run_bass_kernel_spmd(nc, in_maps, core_ids=list(range(N_CORES)),
                               trace=bool(int(os.environ.get("KTRACE", "0"))))
    kernel.last_results = res.results
    out = np.concatenate([res.results[i]["out"] for i in range(N_CORES)], axis=0)
    if bool(int(os.environ.get("KTRACE", "0"))):
        kernel.last_exec_time_ns = res.exec_time_ns
        kernel.last_profile = res.profile_json
    return out.astype(np.float32)


# revision 21
# speedup vs baseline: 1.0551x; 1.0551x over previous
"""Trainium2 Bass kernel for nn_AttentionDecoder (Bahdanau attention + GRU greedy decoder).

Sharding: pure data parallel, B=2048 split as 256 rows per core across 8 cores.
All compute in bf16 with f32 PSUM accumulation.

v2 layout scheme (per core, BL=256):
  - Batch processed as two 128-row halves whose per-step chains are emitted
    phase-interleaved so PE work of one half overlaps ACT/DVE work of the other.
  - ep/att feature-major [a, ab, t, b] (t-major free order so the dec-proj
    broadcast add keeps unit stride for DVE 2x mode); encT built t-major by
    strided-row xbar DMA transposes so the prologue ep copies are contiguous.
  - energy + context via block-diagonal attention matmuls (enc streams through
    PE as stationaries), GRU gates feature-major (weight chunks stationary)
    with every gate bias folded into the always-1 row 37 of onehotT or ones1
    bias matmuls; h stays feature-major forever (no h transposes).
  - sigmoid computed as (1+tanh(x/2))/2 so tanh/exp/relu/copy share one ACT
    table set (no per-step ACT_TABLE_LOAD).
"""

import os
import threading
import numpy as np
import ml_dtypes

N_CORES = 8
B, T, ENC = 2048, 32, 512
DEC, ATT, EMB, NCLS, L = 256, 256, 64, 37, 10
BL = B // N_CORES  # 256 per core

_BF = ml_dtypes.bfloat16

_lock = threading.Lock()
_cache = {}


def _build():
    import concourse.bass as bass
    import concourse.tile as tile
    from concourse import bacc, mybir

    bf = mybir.dt.bfloat16
    f32 = mybir.dt.float32

    nc = bacc.Bacc("TRN2", target_bir_lowering=False, debug=False,
                   num_devices=N_CORES)

    # ---------------- DRAM parameters ----------------
    d_enc = nc.dram_tensor("enc", [BL, T, ENC], bf, kind="ExternalInput").ap()
    d_wdec = nc.dram_tensor("w_dec", [DEC, ATT], bf, kind="ExternalInput").ap()
    d_wenc = nc.dram_tensor("w_enc", [ENC, ATT], bf, kind="ExternalInput").ap()
    d_v = nc.dram_tensor("v", [ATT, 1], bf, kind="ExternalInput").ap()
    d_embWb = nc.dram_tensor("embWb", [NCLS + 1, 3 * DEC], bf, kind="ExternalInput").ap()
    d_wihc = nc.dram_tensor("w_ih_c", [ENC, 3 * DEC], bf, kind="ExternalInput").ap()
    d_whhrz = nc.dram_tensor("w_hh_rz", [DEC, 2 * DEC], bf, kind="ExternalInput").ap()
    d_whhn = nc.dram_tensor("w_hh_n", [DEC, DEC], bf, kind="ExternalInput").ap()
    d_bhhn = nc.dram_tensor("b_hh_n", [1, DEC], bf, kind="ExternalInput").ap()
    d_fc1h = nc.dram_tensor("fc1_w_h", [DEC, DEC], bf, kind="ExternalInput").ap()
    d_fc1c = nc.dram_tensor("fc1_w_c", [ENC, DEC], bf, kind="ExternalInput").ap()
    d_fc1b = nc.dram_tensor("fc1_b", [1, DEC], bf, kind="ExternalInput").ap()
    d_fc2w = nc.dram_tensor("fc2_w", [DEC, NCLS], bf, kind="ExternalInput").ap()
    d_fc2b = nc.dram_tensor("fc2_b", [1, NCLS], bf, kind="ExternalInput").ap()
    d_ihw = nc.dram_tensor("init_h_w", [ENC, DEC], bf, kind="ExternalInput").ap()
    d_ihb = nc.dram_tensor("init_h_b", [DEC, 1], f32, kind="ExternalInput").ap()
    d_out = nc.dram_tensor("out", [BL, L, NCLS], f32, kind="ExternalOutput").ap()
    KDBG = bool(int(os.environ.get("KDBG", "0")))
    if KDBG:
        d_dbg_ep = nc.dram_tensor("dbg_ep", [128, 2, T, 256], bf, kind="ExternalOutput").ap()
        d_dbg_mean = nc.dram_tensor("dbg_mean", [128, 4, BL], bf, kind="ExternalOutput").ap()
        d_dbg_h0 = nc.dram_tensor("dbg_h0", [128, 2, BL], bf, kind="ExternalOutput").ap()
        d_dbg_att = nc.dram_tensor("dbg_att", [128, 2, T, 256], bf, kind="ExternalOutput").ap()
        d_dbg_attn = nc.dram_tensor("dbg_attn", [2, 128, T], f32, kind="ExternalOutput").ap()
        d_dbg_ctx = nc.dram_tensor("dbg_ctx", [128, 4, BL], bf, kind="ExternalOutput").ap()
        d_dbg_h1 = nc.dram_tensor("dbg_h1", [128, 2, BL], bf, kind="ExternalOutput").ap()

    ident_np = np.eye(128, dtype=np.float32)
    d_ident = nc.inline_tensor(ident_np, name="identf").ap()
    d_onesrow = nc.inline_tensor(np.ones((1, BL), dtype=_BF), name="onesrow").ap()

    AluOp = mybir.AluOpType
    ActF = mybir.ActivationFunctionType

    with tile.TileContext(nc) as tc:
        with (
            tc.tile_pool(name="persist", bufs=1) as P,
            tc.tile_pool(name="wpool", bufs=1) as W,
            tc.tile_pool(name="trans", bufs=2) as TR,
            tc.tile_pool(name="small", bufs=2) as SM,
            tc.tile_pool(name="ps", bufs=1, space="PSUM") as PS,
        ):
            # ---------------- persistent SBUF tensors ----------------
            enc_bd = P.tile([128, 64, ENC], bf, tag="enc_bd")       # 64KB/part
            ep = P.tile([128, 2, T, 256], bf, tag="ep")             # enc_proj, t-major
            att = P.tile([128, 2, T, 256], bf, tag="att")           # tanh buffer
            hT = P.tile([128, 2, BL], bf, tag="hT")                 # h feature-major
            ctxT = P.tile([128, 4, BL], bf, tag="ctxT")             # ctx feature-major
            hidT = P.tile([128, 2, BL], bf, tag="hidT")             # fc1 out
            onehotT = P.tile([NCLS + 1, BL], bf, tag="onehotT")
            meanT = P.tile([128, 4, BL], bf, tag="meanT")
            out_sb = P.tile([128, 2, L, NCLS], f32, tag="out_sb")
            ones1 = P.tile([1, BL], bf, tag="ones1")

            # ---------------- weights to SBUF ----------------
            def wload(tag, shape, src, rearr=None):
                t = W.tile(shape, bf, tag=tag, name=tag)
                nc.scalar.dma_start(t[:], src if rearr is None else src.rearrange(rearr, p=128))
                return t

            w_dec = wload("w_dec", [128, 2, ATT], d_wdec, "(k p) n -> p k n")
            w_enc = wload("w_enc", [128, 4, ATT], d_wenc, "(k p) n -> p k n")
            v_sb = wload("v_sb", [128, 2, 1], d_v, "(k p) n -> p k n")
            embWb = wload("embWb", [NCLS + 1, 3 * DEC], d_embWb)
            w_ihc = wload("w_ihc", [128, 4, 3 * DEC], d_wihc, "(k p) n -> p k n")
            w_hhrz = wload("w_hhrz", [128, 2, 2 * DEC], d_whhrz, "(k p) n -> p k n")
            w_hhn = wload("w_hhn", [128, 2, DEC], d_whhn, "(k p) n -> p k n")
            bhhn = wload("bhhn", [1, DEC], d_bhhn)
            fc1h = wload("fc1h", [128, 2, DEC], d_fc1h, "(k p) n -> p k n")
            fc1c = wload("fc1c", [128, 4, DEC], d_fc1c, "(k p) n -> p k n")
            fc1b = wload("fc1b", [1, DEC], d_fc1b)
            fc2w = wload("fc2w", [128, 2, NCLS], d_fc2w, "(k p) n -> p k n")
            fc2b = wload("fc2b", [1, NCLS], d_fc2b)
            ihw = wload("ihw", [128, 4, DEC], d_ihw, "(k p) n -> p k n")
            identf = W.tile([128, 128], f32, tag="identf", name="identf")
            nc.scalar.dma_start(identf[:], d_ident)
            ihb = W.tile([128, 2, 1], f32)
            nc.scalar.dma_start(ihb[:], d_ihb.rearrange("(k p) n -> p k n", p=128))

            nc.vector.memset(ones1[:], 1.0)
            nc.vector.memset(onehotT[0:NCLS, :], 0)
            nc.vector.memset(onehotT[0:1, :], 1.0)
            nc.scalar.dma_start(onehotT[NCLS:NCLS + 1, :], d_onesrow[:])

            # ---------------- prologue ----------------
            # encT built T-MAJOR (cols = (t, b)) by strided-row xbar DMA
            # transposes split across the two HWDGE queues (sync + scalar);
            # ep matmul outputs are then contiguous t-major slabs.
            for bs in range(4):
                nc.scalar.dma_start(
                    enc_bd[32 * bs:32 * bs + 32, :, :],
                    d_enc[bs::4].rearrange("g t e -> t g e"),
                )
            d_enc2d = d_enc.rearrange("b t e -> (b t) e")
            for bth in range(2):
                hs2 = 128 * bth
                encTh = TR.tile([128, 4, 4096], bf, tag="encTh", bufs=1)
                for eb in range(4):
                    eng = nc.sync
                    eng.dma_start_transpose(
                        encTh[:, eb, :],
                        d_enc2d[4096 * bth:4096 * bth + 4096,
                                128 * eb:128 * eb + 128])
                # mean over t (inner reduce); split DVE / Pool
                with nc.allow_low_precision(reason="t-sum of O(1) bf16 values for h0 mean"):
                    for eb in range(4):
                        eng = nc.vector
                        eng.tensor_reduce(
                            meanT[:, eb, hs2:hs2 + 128],
                            encTh[:, eb, :].rearrange("p (b t) -> p b t", t=T),
                            axis=mybir.AxisListType.X, op=AluOp.add)
                # ep matmuls with t-major strided moving operand: each c-slab
                # covers t 4c..4c+3 for all 128 b, so pp copies are contiguous
                for ab in range(2):
                    encT_tm = encTh.rearrange("p k (b t) -> p k t b", t=T)
                    for c in range(8):
                        pp = PS.tile([128, 512], f32, tag="mix", bufs=2)
                        for eb in range(4):
                            nc.tensor.matmul(
                                pp[:],
                                w_enc[:, eb, 128 * ab:128 * ab + 128],
                                encT_tm[:, eb, 4 * c:4 * c + 4, :],
                                start=(eb == 0), stop=(eb == 3),
                            )
                        dst = ep[:, ab, 4 * c:4 * c + 4, hs2:hs2 + 128]
                        src = pp[:].rearrange("p (t b) -> p t b", b=128)
                        if c % 2 == 0:
                            nc.vector.tensor_copy(dst, src)
                        else:
                            nc.scalar.copy(dst, src)

            # h0 = tanh(mean_enc @ (init_h_W/32) + b)   (feature-major)
            for db in range(2):
                hp = PS.tile([128, BL], f32, tag="cp", bufs=1)
                for eb in range(4):
                    nc.tensor.matmul(hp[:], ihw[:, eb, 128 * db:128 * db + 128],
                                     meanT[:, eb, :], start=(eb == 0), stop=(eb == 3))
                nc.scalar.activation(hT[:, db, :], hp[:], ActF.Tanh, bias=ihb[:, db, :])

            if KDBG:
                nc.sync.dma_start(d_dbg_ep, ep[:])
                nc.sync.dma_start(d_dbg_mean, meanT[:])
                nc.sync.dma_start(d_dbg_h0, hT[:])

            # ---------------- decode loop (software-pipelined) ----------------
            # The fc1/fc2/argmax tail of step s-1 is emitted at the start of
            # iteration s so it fills the PE pipe while step s's add/tanh runs
            # on DVE/ACT; energy pairs interleave with tanh t-halves so PE
            # stays continuously busy (keeps the high p-state clock).

            def fc_tail(pstep, pmixes):
                # fc1 full-batch feature-major; reads post-GRU hT and the
                # previous step's ctxT (still live; ctx(s) overwrites later).
                fp = PS.tile([128, 2, BL], f32, tag="cp", bufs=1)
                for dc in range(2):
                    ds = 128 * dc
                    nc.tensor.matmul(fp[:, dc, :], fc1b[:, ds:ds + 128],
                                     ones1[:], start=True, stop=False)
                    for db in range(2):
                        nc.tensor.matmul(fp[:, dc, :], fc1h[:, db, ds:ds + 128],
                                         hT[:, db, :], start=False, stop=False)
                    for eb in range(4):
                        nc.tensor.matmul(fp[:, dc, :], fc1c[:, eb, ds:ds + 128],
                                         ctxT[:, eb, :], start=False,
                                         stop=(eb == 3))
                nc.scalar.activation(hidT[:], fp[:], ActF.Relu)
                for half in range(2):
                    hs, he = 128 * half, 128 * half + 128
                    mix = pmixes[half]
                    nc.tensor.matmul(mix[:, 256:256 + NCLS], ones1[:, hs:he],
                                     fc2b[:], start=True, stop=False)
                    for db in range(2):
                        nc.tensor.matmul(mix[:, 256:256 + NCLS],
                                         hidT[:, db, hs:he], fc2w[:, db, :],
                                         start=False, stop=(db == 1))
                    nc.scalar.copy(out_sb[:, half, pstep, :],
                                   mix[:, 256:256 + NCLS])
                    if pstep < L - 1:
                        mx = SM.tile([128, 1], f32, tag="mx")
                        nc.vector.tensor_reduce(mx[:], mix[:, 256:256 + NCLS],
                                                axis=mybir.AxisListType.X,
                                                op=AluOp.max)
                        ohB = SM.tile([128, NCLS], f32, tag="ohB")
                        nc.vector.tensor_tensor(
                            ohB[:], mix[:, 256:256 + NCLS],
                            mx[:].broadcast_to([128, NCLS]), op=AluOp.is_equal)
                        nc.tensor.transpose(mix[0:NCLS, 320:448], ohB[:],
                                            identf[:])
                        nc.vector.tensor_copy(onehotT[0:NCLS, hs:he],
                                              mix[0:NCLS, 320:448])

            prev_mixes = None
            for step in range(L):
                # dec_proj feature-major: dp[ab] = W_dec[:,ab]^T h   [a, b]
                dp = PS.tile([128, 2, BL], f32, tag="dp", bufs=1)
                for ab in range(2):
                    for db in range(2):
                        nc.tensor.matmul(dp[:, ab, :],
                                         w_dec[:, db, 128 * ab:128 * ab + 128],
                                         hT[:, db, :], start=(db == 0),
                                         stop=(db == 1))
                if step > 0:
                    fc_tail(step - 1, prev_mixes)
                decT = SM.tile([128, 2, BL], bf, tag="decT")
                for ab in range(2):
                    nc.vector.tensor_copy(decT[:, ab, :], dp[:, ab, :])

                # add + tanh waves, energy pairs chasing each tanh t-half
                mixes = [None, None]
                for half in range(2):
                    hs, he = 128 * half, 128 * half + 128
                    mix = PS.tile([128, 512], f32, tag="mix", bufs=2)
                    mixes[half] = mix
                    for th in range(2):
                        ts = 16 * th
                        for ab in range(2):
                            bcast = decT[:, ab, hs:he].rearrange(
                                "p (o b) -> p o b", o=1).broadcast_to(
                                    [128, 16, 128])
                            nc.vector.tensor_tensor(
                                att[:, ab, ts:ts + 16, hs:he],
                                ep[:, ab, ts:ts + 16, hs:he], bcast,
                                op=AluOp.add)
                            nc.scalar.activation(att[:, ab, ts:ts + 16, hs:he],
                                                 att[:, ab, ts:ts + 16, hs:he],
                                                 ActF.Tanh)
                        for t in range(ts, ts + 16):
                            for ab in range(2):
                                nc.tensor.matmul(mix[:, t:t + 1],
                                                 att[:, ab, t, hs:he],
                                                 v_sb[:, ab, :],
                                                 start=(ab == 0), stop=(ab == 1))

                # softmax + attn transpose + blockdiag + context per half
                for half in range(2):
                    hs, he = 128 * half, 128 * half + 128
                    mix = mixes[half]
                    expB = SM.tile([128, T], f32, tag="expB")
                    zc = SM.tile([128, 1], f32, tag="zc")
                    nc.scalar.activation(expB[:], mix[:, 0:T], ActF.Exp,
                                         accum_out=zc[:])
                    rz = SM.tile([128, 1], f32, tag="rz")
                    nc.vector.reciprocal(rz[:], zc[:])
                    attnB = SM.tile([128, T], f32, tag="attnB")
                    nc.vector.tensor_scalar(attnB[:], expB[:], rz[:], None,
                                            op0=AluOp.mult)
                    if KDBG and step == 0:
                        nc.sync.dma_start(d_dbg_attn[half], attnB[:])
                    nc.tensor.transpose(mix[0:T, 128:256], attnB[:], identf[:])
                    attn_bd = SM.tile([128, 32, 4], bf, tag="attn_bd")
                    nc.vector.memset(attn_bd[:], 0)
                    for bs in range(4):
                        nc.vector.tensor_copy(
                            attn_bd[32 * bs:32 * bs + 32, :, bs],
                            mix[0:T, 128 + bs:256:4],
                        )
                    cpt = PS.tile([128, 4, 128], f32, tag="cp", bufs=1)
                    for eb in range(4):
                        for gr in range(32):
                            g = 32 * half + gr
                            nc.tensor.matmul(cpt[:, eb, 4 * gr:4 * gr + 4],
                                             enc_bd[:, g, 128 * eb:128 * eb + 128],
                                             attn_bd[:, gr, :], start=True,
                                             stop=True)
                    for eb in range(4):
                        nc.vector.tensor_copy(ctxT[:, eb, hs:he], cpt[:, eb, :])

                if KDBG and step == 0:
                    nc.sync.dma_start(d_dbg_att, att[:])
                    nc.sync.dma_start(d_dbg_ctx, ctxT[:])

                # GRU gates, full batch, feature-major (weight chunks stationary)
                oh = onehotT[:]
                rzp = PS.tile([128, 4, BL], f32, tag="rzp", bufs=1)
                ngp = PS.tile([128, 4, BL], f32, tag="ngp", bufs=1)
                for fc in range(4):
                    fs = 128 * fc
                    nc.tensor.matmul(rzp[:, fc, :], embWb[:, fs:fs + 128], oh,
                                     start=True, stop=False)
                    for eb in range(4):
                        nc.tensor.matmul(rzp[:, fc, :],
                                         w_ihc[:, eb, fs:fs + 128],
                                         ctxT[:, eb, :], start=False, stop=False)
                    for db in range(2):
                        nc.tensor.matmul(rzp[:, fc, :],
                                         w_hhrz[:, db, fs:fs + 128],
                                         hT[:, db, :], start=False,
                                         stop=(db == 1))
                for fc in range(2):
                    fs = 128 * fc
                    # n-pre: i_n (emb bias row included) + ctx + 0.5*hn + 0.5*bhhn
                    nc.tensor.matmul(ngp[:, fc, :],
                                     embWb[:, 512 + fs:512 + fs + 128], oh,
                                     start=True, stop=False)
                    for eb in range(4):
                        nc.tensor.matmul(ngp[:, fc, :],
                                         w_ihc[:, eb, 512 + fs:512 + fs + 128],
                                         ctxT[:, eb, :], start=False, stop=False)
                    nc.tensor.matmul(ngp[:, fc, :], bhhn[:, fs:fs + 128],
                                     ones1[:], start=False, stop=False)
                    for db in range(2):
                        nc.tensor.matmul(ngp[:, fc, :],
                                         w_hhn[:, db, fs:fs + 128],
                                         hT[:, db, :], start=False,
                                         stop=(db == 1))
                    # ghn = 0.5*hn + 0.5*bhhn
                    nc.tensor.matmul(ngp[:, 2 + fc, :], bhhn[:, fs:fs + 128],
                                     ones1[:], start=True, stop=False)
                    for db in range(2):
                        nc.tensor.matmul(ngp[:, 2 + fc, :],
                                         w_hhn[:, db, fs:fs + 128],
                                         hT[:, db, :], start=False,
                                         stop=(db == 1))

                # GRU elementwise, full batch, feature-major; h in place
                tr = SM.tile([128, 2, BL], bf, tag="tr")
                nc.scalar.activation(tr[:], rzp[:, 0:2, :], ActF.Tanh, scale=0.5)
                tz = SM.tile([128, 2, BL], bf, tag="tz")
                nc.scalar.activation(tz[:], rzp[:, 2:4, :], ActF.Tanh, scale=0.5)
                rhn = SM.tile([128, 2, BL], bf, tag="rhn")
                nc.vector.tensor_tensor(rhn[:], tr[:], ngp[:, 2:4, :],
                                        op=AluOp.mult)
                npre = SM.tile([128, 2, BL], bf, tag="npre")
                nc.vector.tensor_tensor(npre[:], ngp[:, 0:2, :], rhn[:],
                                        op=AluOp.add)
                nn_ = SM.tile([128, 2, BL], bf, tag="nn_")
                nc.scalar.activation(nn_[:], npre[:], ActF.Tanh)
                t1 = SM.tile([128, 2, BL], bf, tag="t1")
                nc.vector.tensor_tensor(t1[:], hT[:], nn_[:], op=AluOp.subtract)
                t2 = SM.tile([128, 2, BL], bf, tag="t2")
                nc.vector.scalar_tensor_tensor(t2[:], tz[:], 1.0, t1[:],
                                               op0=AluOp.add, op1=AluOp.mult)
                nc.vector.scalar_tensor_tensor(hT[:], t2[:], 0.5, nn_[:],
                                               op0=AluOp.mult, op1=AluOp.add)
                if KDBG and step == 0:
                    nc.sync.dma_start(d_dbg_h1, hT[:])
                prev_mixes = mixes

            fc_tail(L - 1, prev_mixes)

            # ---------------- output DMA ----------------
            for half in range(2):
                nc.sync.dma_start(
                    d_out[128 * half:128 * half + 128],
                    out_sb[:, half, :, :],
                )

    nc.compile()
    return nc


def _get_nc():
    with _lock:
        if "nc" not in _cache:
            _cache["nc"] = _build()
        return _cache["nc"]


def kernel(**inputs):
    nc = _get_nc()
    from concourse.bass_utils import run_bass_kernel_spmd

    enc = np.ascontiguousarray(inputs["encoder_outputs"], dtype=np.float32)
    emb = inputs["emb"].astype(np.float32)
    W_enc = inputs["W_enc"].astype(np.float32)
    W_dec = inputs["W_dec"].astype(np.float32)
    v = inputs["v"].astype(np.float32)
    init_h_W = inputs["init_h_W"].astype(np.float32)
    init_h_b = inputs["init_h_b"].astype(np.float32)
    W_ih = inputs["W_ih"].astype(np.float32)
    b_ih = inputs["b_ih"].astype(np.float32)
    W_hh = inputs["W_hh"].astype(np.float32)
    b_hh = inputs["b_hh"].astype(np.float32)
    fc1_W = inputs["fc1_W"].astype(np.float32)
    fc1_b = inputs["fc1_b"].astype(np.float32)
    fc2_W = inputs["fc2_W"].astype(np.float32)
    fc2_b = inputs["fc2_b"].astype(np.float32)

    # host precompute: embedding projected through W_ih (emb part) + rz biases;
    # W_hh_n/b_hh_n halved for the tanh-form sigmoid r-gate; init_h_W absorbs
    # the 1/T mean divisor.
    bias_row = np.concatenate([(b_ih + b_hh)[:2 * DEC], b_ih[2 * DEC:]])
    embWb = np.concatenate([emb @ W_ih[:EMB], bias_row[None, :]], axis=0)

    bfc = lambda a: np.ascontiguousarray(a, dtype=_BF)
    shared = {
        "w_dec": bfc(W_dec),
        "w_enc": bfc(W_enc),
        "v": bfc(v.reshape(ATT, 1)),
        "embWb": bfc(embWb),
        "w_ih_c": bfc(W_ih[EMB:]),
        "w_hh_rz": bfc(W_hh[:, :2 * DEC]),
        "w_hh_n": bfc(0.5 * W_hh[:, 2 * DEC:]),
        "b_hh_n": bfc(0.5 * b_hh[2 * DEC:].reshape(1, DEC)),
        "fc1_w_h": bfc(fc1_W[:DEC]),
        "fc1_w_c": bfc(fc1_W[DEC:]),
        "fc1_b": bfc(fc1_b.reshape(1, DEC)),
        "fc2_w": bfc(fc2_W),
        "fc2_b": bfc(fc2_b.reshape(1, NCLS)),
        "init_h_w": bfc(init_h_W / T),
        "init_h_b": np.ascontiguousarray(init_h_b.reshape(DEC, 1), dtype=np.float32),
    }
    enc_bf = enc.astype(_BF)
    in_maps = []
    for i in range(N_CORES):
        m = dict(shared)
        m["enc"] = np.ascontiguousarray(enc_bf[i * BL:(i + 1) * BL])
        in_maps.append(m)

    res = run_bass_kernel_spmd(nc, in_maps, core_ids=list(range(N_CORES)),
                               trace=bool(int(os.environ.get("KTRACE", "0"))))
    kernel.last_results = res.results
    out = np.concatenate([res.results[i]["out"] for i in range(N_CORES)], axis=0)
    if bool(int(os.environ.get("KTRACE", "0"))):
        kernel.last_exec_time_ns = res.exec_time_ns
        kernel.last_profile = res.profile_json
    return out.astype(np.float32)


# revision 26
# speedup vs baseline: 1.3298x; 1.2603x over previous
"""Trainium2 Bass kernel for nn_AttentionDecoder (Bahdanau attention + GRU greedy decoder).

Sharding: pure data parallel, B=2048 split as 256 rows per core across 8 cores.
All compute in bf16 with f32 PSUM accumulation.

v2 layout scheme (per core, BL=256):
  - Batch processed as two 128-row halves whose per-step chains are emitted
    phase-interleaved so PE work of one half overlaps ACT/DVE work of the other.
  - ep/att feature-major [a, ab, t, b] (t-major free order so the dec-proj
    broadcast add keeps unit stride for DVE 2x mode); encT built t-major by
    strided-row xbar DMA transposes so the prologue ep copies are contiguous.
  - energy + context via block-diagonal attention matmuls (enc streams through
    PE as stationaries), GRU gates feature-major (weight chunks stationary)
    with every gate bias folded into the always-1 row 37 of onehotT or ones1
    bias matmuls; h stays feature-major forever (no h transposes).
  - sigmoid computed as (1+tanh(x/2))/2 so tanh/exp/relu/copy share one ACT
    table set (no per-step ACT_TABLE_LOAD).
"""

import os
import threading
import numpy as np
import ml_dtypes

N_CORES = 8
B, T, ENC = 2048, 32, 512
DEC, ATT, EMB, NCLS, L = 256, 256, 64, 37, 10
BL = B // N_CORES  # 256 per core

_BF = ml_dtypes.bfloat16

_lock = threading.Lock()
_cache = {}


def _build():
    import concourse.bass as bass
    import concourse.tile as tile
    from concourse import bacc, mybir

    bf = mybir.dt.bfloat16
    f32 = mybir.dt.float32

    nc = bacc.Bacc("TRN2", target_bir_lowering=False, debug=False,
                   num_devices=N_CORES)

    # ---------------- DRAM parameters ----------------
    f8 = mybir.dt.float8e4
    d_enc = nc.dram_tensor("enc", [2, T, 128, ENC], bf, kind="ExternalInput").ap()
    d_enc8 = nc.dram_tensor("enc8", [2, T, 128, ENC], f8, kind="ExternalInput").ap()
    d_v8 = nc.dram_tensor("v8", [ATT, 1], f8, kind="ExternalInput").ap()
    d_wdec = nc.dram_tensor("w_dec", [DEC, ATT], bf, kind="ExternalInput").ap()
    d_wenc = nc.dram_tensor("w_enc", [ENC, ATT], bf, kind="ExternalInput").ap()
    d_v = nc.dram_tensor("v", [ATT, 1], bf, kind="ExternalInput").ap()
    d_embWb = nc.dram_tensor("embWb", [NCLS + 1, 3 * DEC], bf, kind="ExternalInput").ap()
    d_wihc = nc.dram_tensor("w_ih_c", [ENC, 3 * DEC], bf, kind="ExternalInput").ap()
    d_whhrz = nc.dram_tensor("w_hh_rz", [DEC, 2 * DEC], bf, kind="ExternalInput").ap()
    d_whhn = nc.dram_tensor("w_hh_n", [DEC, DEC], bf, kind="ExternalInput").ap()
    d_bhhn = nc.dram_tensor("b_hh_n", [1, DEC], bf, kind="ExternalInput").ap()
    d_fc1h = nc.dram_tensor("fc1_w_h", [DEC, DEC], bf, kind="ExternalInput").ap()
    d_fc1c = nc.dram_tensor("fc1_w_c", [ENC, DEC], bf, kind="ExternalInput").ap()
    d_fc1b = nc.dram_tensor("fc1_b", [1, DEC], bf, kind="ExternalInput").ap()
    d_fc2w = nc.dram_tensor("fc2_w", [DEC, NCLS], bf, kind="ExternalInput").ap()
    d_fc2b = nc.dram_tensor("fc2_b", [1, NCLS], bf, kind="ExternalInput").ap()
    d_ihw = nc.dram_tensor("init_h_w", [ENC, DEC], bf, kind="ExternalInput").ap()
    d_ihb = nc.dram_tensor("init_h_b", [DEC, 1], f32, kind="ExternalInput").ap()
    d_out = nc.dram_tensor("out", [BL, L, NCLS], f32, kind="ExternalOutput").ap()
    KDBG = bool(int(os.environ.get("KDBG", "0")))
    if KDBG:
        d_dbg_ep = nc.dram_tensor("dbg_ep", [128, 2, T, 256], bf, kind="ExternalOutput").ap()
        d_dbg_mean = nc.dram_tensor("dbg_mean", [128, 4, BL], bf, kind="ExternalOutput").ap()
        d_dbg_h0 = nc.dram_tensor("dbg_h0", [128, 2, BL], bf, kind="ExternalOutput").ap()
        d_dbg_att = nc.dram_tensor("dbg_att", [128, 2, T, 256], bf, kind="ExternalOutput").ap()
        d_dbg_attn = nc.dram_tensor("dbg_attn", [2, 128, T], f32, kind="ExternalOutput").ap()
        d_dbg_ctx = nc.dram_tensor("dbg_ctx", [128, 4, BL], bf, kind="ExternalOutput").ap()
        d_dbg_h1 = nc.dram_tensor("dbg_h1", [128, 2, BL], bf, kind="ExternalOutput").ap()

    ident_np = np.eye(128, dtype=np.float32)
    d_ident = nc.inline_tensor(ident_np, name="identf").ap()
    d_onesrow = nc.inline_tensor(np.ones((1, BL), dtype=_BF), name="onesrow").ap()

    AluOp = mybir.AluOpType
    ActF = mybir.ActivationFunctionType

    with tile.TileContext(nc) as tc:
        with (
            tc.tile_pool(name="persist", bufs=1) as P,
            tc.tile_pool(name="wpool", bufs=1) as W,
            tc.tile_pool(name="trans", bufs=2) as TR,
            tc.tile_pool(name="small", bufs=2) as SM,
            tc.tile_pool(name="ps", bufs=1, space="PSUM") as PS,
        ):
            # ---------------- persistent SBUF tensors ----------------
            enc_bd = P.tile([128, 64, ENC], bf, tag="enc_bd")       # 64KB/part
            ep = P.tile([128, 2, T, 256], bf, tag="ep")             # enc_proj, t-major
            hT = P.tile([128, 2, BL], bf, tag="hT")                 # h feature-major
            ctxT = P.tile([128, 4, BL], bf, tag="ctxT")             # ctx feature-major
            hidT = P.tile([128, 2, BL], bf, tag="hidT")             # fc1 out
            onehotT = P.tile([NCLS + 1, BL], bf, tag="onehotT")
            meanT = P.tile([128, 4, BL], bf, tag="meanT")
            out_sb = P.tile([128, 2, L, NCLS], f32, tag="out_sb")
            ones1 = P.tile([1, BL], bf, tag="ones1")

            # ---------------- weights to SBUF ----------------
            def wload(tag, shape, src, rearr=None):
                t = W.tile(shape, bf, tag=tag, name=tag)
                nc.scalar.dma_start(t[:], src if rearr is None else src.rearrange(rearr, p=128))
                return t

            w_dec = wload("w_dec", [128, 2, ATT], d_wdec, "(k p) n -> p k n")
            w_enc = wload("w_enc", [128, 4, ATT], d_wenc, "(k p) n -> p k n")
            v_sb = wload("v_sb", [128, 2, 1], d_v, "(k p) n -> p k n")
            embWb = wload("embWb", [NCLS + 1, 3 * DEC], d_embWb)
            w_ihc = wload("w_ihc", [128, 4, 3 * DEC], d_wihc, "(k p) n -> p k n")
            w_hhrz = wload("w_hhrz", [128, 2, 2 * DEC], d_whhrz, "(k p) n -> p k n")
            w_hhn = wload("w_hhn", [128, 2, DEC], d_whhn, "(k p) n -> p k n")
            bhhn = wload("bhhn", [1, DEC], d_bhhn)
            fc1h = wload("fc1h", [128, 2, DEC], d_fc1h, "(k p) n -> p k n")
            fc1c = wload("fc1c", [128, 4, DEC], d_fc1c, "(k p) n -> p k n")
            fc1b = wload("fc1b", [1, DEC], d_fc1b)
            fc2w = wload("fc2w", [128, 2, NCLS], d_fc2w, "(k p) n -> p k n")
            fc2b = wload("fc2b", [1, NCLS], d_fc2b)
            ihw = wload("ihw", [128, 4, DEC], d_ihw, "(k p) n -> p k n")
            identf = W.tile([128, 128], f32, tag="identf", name="identf")
            nc.scalar.dma_start(identf[:], d_ident)
            ihb = W.tile([128, 2, 1], f32)
            nc.scalar.dma_start(ihb[:], d_ihb.rearrange("(k p) n -> p k n", p=128))

            nc.vector.memset(ones1[:], 1.0)
            nc.vector.memset(onehotT[0:NCLS, :], 0)
            nc.vector.memset(onehotT[0:1, :], 1.0)
            nc.scalar.dma_start(onehotT[NCLS:NCLS + 1, :], d_onesrow[:])

            # ---------------- prologue ----------------
            # encT built T-MAJOR (cols = (t, b)) by strided-row xbar DMA
            # transposes split across the two HWDGE queues (sync + scalar);
            # ep matmul outputs are then contiguous t-major slabs.
            # enc_bd[32*bs+t, g, e] = enc[4*g+bs, t, e]  (g global over both halves)
            for bs in range(4):
                for bth in range(2):
                    nc.scalar.dma_start(
                        enc_bd[32 * bs:32 * bs + 32, 32 * bth:32 * bth + 32, :],
                        d_enc[bth, :, bs::4, :])
            # host sends enc t-major per 128-half, so the xbar source and the
            # ep matmul moving slabs are both contiguous
            for bth in range(2):
                hs2 = 128 * bth
                d_encb = d_enc[bth].rearrange("t b e -> (t b) e")
                encTh = TR.tile([128, 4, 4096], bf, tag="encTh", bufs=1)
                for eb in range(4):
                    nc.sync.dma_start_transpose(
                        encTh[:, eb, :],
                        d_encb[:, 128 * eb:128 * eb + 128])
                # mean over t (outer axis -> strided inner reduce)
                with nc.allow_low_precision(reason="t-sum of O(1) bf16 values for h0 mean"):
                    for eb in range(4):
                        nc.vector.tensor_reduce(
                            meanT[:, eb, hs2:hs2 + 128],
                            encTh[:, eb, :].rearrange("p (t b) -> p b t", b=128),
                            axis=mybir.AxisListType.X, op=AluOp.add)
                for ab in range(2):
                    for c in range(8):
                        pp = PS.tile([128, 512], f32, tag="mix", bufs=2)
                        for eb in range(4):
                            nc.tensor.matmul(
                                pp[:],
                                w_enc[:, eb, 128 * ab:128 * ab + 128],
                                encTh[:, eb, 512 * c:512 * c + 512],
                                start=(eb == 0), stop=(eb == 3),
                            )
                        dst = ep[:, ab, 4 * c:4 * c + 4, hs2:hs2 + 128]
                        src = pp[:].rearrange("p (t b) -> p t b", b=128)
                        if c % 2 == 0:
                            nc.vector.tensor_copy(dst, src)
                        else:
                            nc.scalar.copy(dst, src)

            # h0 = tanh(mean_enc @ (init_h_W/32) + b)   (feature-major)
            for db in range(2):
                hp = PS.tile([128, BL], f32, tag="cp", bufs=1)
                for eb in range(4):
                    nc.tensor.matmul(hp[:], ihw[:, eb, 128 * db:128 * db + 128],
                                     meanT[:, eb, :], start=(eb == 0), stop=(eb == 3))
                nc.scalar.activation(hT[:, db, :], hp[:], ActF.Tanh, bias=ihb[:, db, :])

            # att reuses the (now dead) encTh transpose buffer
            att = TR.tile([128, 2, T, 256], bf, tag="encTh", bufs=1, name="att")

            if KDBG:
                nc.sync.dma_start(d_dbg_ep, ep[:])
                nc.sync.dma_start(d_dbg_mean, meanT[:])
                nc.sync.dma_start(d_dbg_h0, hT[:])

            # ---------------- decode loop (software-pipelined) ----------------
            # The fc1/fc2/argmax tail of step s-1 is emitted at the start of
            # iteration s so it fills the PE pipe while step s's add/tanh runs
            # on DVE/ACT; energy pairs interleave with tanh t-halves so PE
            # stays continuously busy (keeps the high p-state clock).

            def fc_tail(pstep, pmixes):
                # fc1 full-batch feature-major; reads post-GRU hT and the
                # previous step's ctxT (still live; ctx(s) overwrites later).
                fp = PS.tile([128, 2, BL], f32, tag="cp", bufs=1)
                for dc in range(2):
                    ds = 128 * dc
                    nc.tensor.matmul(fp[:, dc, :], fc1b[:, ds:ds + 128],
                                     ones1[:], start=True, stop=False)
                    for db in range(2):
                        nc.tensor.matmul(fp[:, dc, :], fc1h[:, db, ds:ds + 128],
                                         hT[:, db, :], start=False, stop=False)
                    for eb in range(4):
                        nc.tensor.matmul(fp[:, dc, :], fc1c[:, eb, ds:ds + 128],
                                         ctxT[:, eb, :], start=False,
                                         stop=(eb == 3))
                nc.scalar.activation(hidT[:], fp[:], ActF.Relu)
                for half in range(2):
                    hs, he = 128 * half, 128 * half + 128
                    mix = pmixes[half]
                    nc.tensor.matmul(mix[:, 256:256 + NCLS], ones1[:, hs:he],
                                     fc2b[:], start=True, stop=False)
                    for db in range(2):
                        nc.tensor.matmul(mix[:, 256:256 + NCLS],
                                         hidT[:, db, hs:he], fc2w[:, db, :],
                                         start=False, stop=(db == 1))
                    nc.scalar.copy(out_sb[:, half, pstep, :],
                                   mix[:, 256:256 + NCLS])
                    nc.sync.dma_start(d_out[128 * half:128 * half + 128,
                                            pstep, :],
                                      out_sb[:, half, pstep, :])
                    if pstep < L - 1:
                        mx = SM.tile([128, 1], f32, tag="mx")
                        nc.vector.tensor_reduce(mx[:], mix[:, 256:256 + NCLS],
                                                axis=mybir.AxisListType.X,
                                                op=AluOp.max)
                        ohB = SM.tile([128, NCLS], f32, tag="ohB")
                        nc.vector.tensor_tensor(
                            ohB[:], mix[:, 256:256 + NCLS],
                            mx[:].broadcast_to([128, NCLS]), op=AluOp.is_equal)
                        nc.tensor.transpose(mix[0:NCLS, 320:448], ohB[:],
                                            identf[:])
                        nc.vector.tensor_copy(onehotT[0:NCLS, hs:he],
                                              mix[0:NCLS, 320:448])

            prev_mixes = None
            for step in range(L):
                # dec_proj feature-major: dp[ab] = W_dec[:,ab]^T h   [a, b]
                dp = PS.tile([128, 2, BL], f32, tag="dp", bufs=1)
                for ab in range(2):
                    for db in range(2):
                        nc.tensor.matmul(dp[:, ab, :],
                                         w_dec[:, db, 128 * ab:128 * ab + 128],
                                         hT[:, db, :], start=(db == 0),
                                         stop=(db == 1))
                if step > 0:
                    fc_tail(step - 1, prev_mixes)
                decT = SM.tile([128, 2, BL], bf, tag="decT", bufs=1)
                for ab in range(2):
                    nc.vector.tensor_copy(decT[:, ab, :], dp[:, ab, :])

                # add + tanh (fp8 out), then DoubleRow energy: one pair per t
                mixes = [None, None]
                for half in range(2):
                    hs, he = 128 * half, 128 * half + 128
                    mix = PS.tile([128, 512], f32, tag="mix", bufs=2)
                    mixes[half] = mix
                    for ab in range(2):
                        bcast = decT[:, ab, hs:he].rearrange(
                            "p (o b) -> p o b", o=1).broadcast_to(
                                [128, T, 128])
                        nc.vector.tensor_tensor(
                            att[:, ab, :, hs:he],
                            ep[:, ab, :, hs:he], bcast, op=AluOp.add)
                        nc.scalar.activation(att[:, ab, :, hs:he],
                                             att[:, ab, :, hs:he], ActF.Tanh)
                    for t in range(T):
                        for ab in range(2):
                            nc.tensor.matmul(mix[:, t:t + 1],
                                             att[:, ab, t, hs:he],
                                             v_sb[:, ab, :],
                                             start=(ab == 0), stop=(ab == 1))

                # softmax + attn transpose + blockdiag + context per half
                for half in range(2):
                    hs, he = 128 * half, 128 * half + 128
                    mix = mixes[half]
                    expB = SM.tile([128, T], f32, tag="expB")
                    zc = SM.tile([128, 1], f32, tag="zc")
                    nc.scalar.activation(expB[:], mix[:, 0:T], ActF.Exp,
                                         accum_out=zc[:])
                    rz = SM.tile([128, 1], f32, tag="rz")
                    nc.vector.reciprocal(rz[:], zc[:])
                    attnB = SM.tile([128, T], f32, tag="attnB")
                    nc.vector.tensor_scalar(attnB[:], expB[:], rz[:], None,
                                            op0=AluOp.mult)
                    if KDBG and step == 0:
                        nc.sync.dma_start(d_dbg_attn[half], attnB[:])
                    nc.tensor.transpose(mix[0:T, 128:256], attnB[:], identf[:])
                    attn_bd = SM.tile([128, 32, 4], bf, tag="attn_bd")
                    nc.vector.memset(attn_bd[:], 0)
                    for bs in range(4):
                        nc.vector.tensor_copy(
                            attn_bd[32 * bs:32 * bs + 32, :, bs],
                            mix[0:T, 128 + bs:256:4],
                        )
                    cpt = PS.tile([128, 4, 128], f32, tag="cp", bufs=1)
                    for eb in range(4):
                        for gr in range(32):
                            g = 32 * half + gr
                            nc.tensor.matmul(cpt[:, eb, 4 * gr:4 * gr + 4],
                                             enc_bd[:, g, 128 * eb:128 * eb + 128],
                                             attn_bd[:, gr, :], start=True,
                                             stop=True)
                    for eb in range(4):
                        nc.vector.tensor_copy(ctxT[:, eb, hs:he], cpt[:, eb, :])

                if KDBG and step == 0:
                    nc.sync.dma_start(d_dbg_ctx, ctxT[:])

                # GRU gates, full batch, feature-major (weight chunks stationary)
                oh = onehotT[:]
                rzp = PS.tile([128, 4, BL], f32, tag="rzp", bufs=1)
                ngp = PS.tile([128, 4, BL], f32, tag="ngp", bufs=1)
                for fc in range(4):
                    fs = 128 * fc
                    nc.tensor.matmul(rzp[:, fc, :], embWb[:, fs:fs + 128], oh,
                                     start=True, stop=False)
                    for eb in range(4):
                        nc.tensor.matmul(rzp[:, fc, :],
                                         w_ihc[:, eb, fs:fs + 128],
                                         ctxT[:, eb, :], start=False, stop=False)
                    for db in range(2):
                        nc.tensor.matmul(rzp[:, fc, :],
                                         w_hhrz[:, db, fs:fs + 128],
                                         hT[:, db, :], start=False,
                                         stop=(db == 1))
                for fc in range(2):
                    fs = 128 * fc
                    # n-pre: i_n (emb bias row included) + ctx + 0.5*hn + 0.5*bhhn
                    nc.tensor.matmul(ngp[:, fc, :],
                                     embWb[:, 512 + fs:512 + fs + 128], oh,
                                     start=True, stop=False)
                    for eb in range(4):
                        nc.tensor.matmul(ngp[:, fc, :],
                                         w_ihc[:, eb, 512 + fs:512 + fs + 128],
                                         ctxT[:, eb, :], start=False, stop=False)
                    nc.tensor.matmul(ngp[:, fc, :], bhhn[:, fs:fs + 128],
                                     ones1[:], start=False, stop=False)
                    for db in range(2):
                        nc.tensor.matmul(ngp[:, fc, :],
                                         w_hhn[:, db, fs:fs + 128],
                                         hT[:, db, :], start=False,
                                         stop=(db == 1))
                    # ghn = 0.5*hn + 0.5*bhhn
                    nc.tensor.matmul(ngp[:, 2 + fc, :], bhhn[:, fs:fs + 128],
                                     ones1[:], start=True, stop=False)
                    for db in range(2):
                        nc.tensor.matmul(ngp[:, 2 + fc, :],
                                         w_hhn[:, db, fs:fs + 128],
                                         hT[:, db, :], start=False,
                                         stop=(db == 1))

                # GRU elementwise, full batch, feature-major; h in place
                tr = SM.tile([128, 2, BL], bf, tag="tr", bufs=1)
                nc.scalar.activation(tr[:], rzp[:, 0:2, :], ActF.Tanh, scale=0.5)
                tz = SM.tile([128, 2, BL], bf, tag="tz", bufs=1)
                nc.scalar.activation(tz[:], rzp[:, 2:4, :], ActF.Tanh, scale=0.5)
                rhn = SM.tile([128, 2, BL], bf, tag="rhn", bufs=1)
                nc.vector.tensor_tensor(rhn[:], tr[:], ngp[:, 2:4, :],
                                        op=AluOp.mult)
                npre = SM.tile([128, 2, BL], bf, tag="npre", bufs=1)
                nc.vector.tensor_tensor(npre[:], ngp[:, 0:2, :], rhn[:],
                                        op=AluOp.add)
                nn_ = SM.tile([128, 2, BL], bf, tag="nn_", bufs=1)
                nc.scalar.activation(nn_[:], npre[:], ActF.Tanh)
                t1 = SM.tile([128, 2, BL], bf, tag="t1", bufs=1)
                nc.vector.tensor_tensor(t1[:], hT[:], nn_[:], op=AluOp.subtract)
                t2 = SM.tile([128, 2, BL], bf, tag="t2", bufs=1)
                nc.vector.scalar_tensor_tensor(t2[:], tz[:], 1.0, t1[:],
                                               op0=AluOp.add, op1=AluOp.mult)
                nc.vector.scalar_tensor_tensor(hT[:], t2[:], 0.5, nn_[:],
                                               op0=AluOp.mult, op1=AluOp.add)
                if KDBG and step == 0:
                    nc.sync.dma_start(d_dbg_h1, hT[:])
                prev_mixes = mixes

            fc_tail(L - 1, prev_mixes)



    nc.compile()
    return nc


def _get_nc():
    with _lock:
        if "nc" not in _cache:
            _cache["nc"] = _build()
        return _cache["nc"]


def kernel(**inputs):
    nc = _get_nc()
    from concourse.bass_utils import run_bass_kernel_spmd

    enc = np.ascontiguousarray(inputs["encoder_outputs"], dtype=np.float32)
    emb = inputs["emb"].astype(np.float32)
    W_enc = inputs["W_enc"].astype(np.float32)
    W_dec = inputs["W_dec"].astype(np.float32)
    v = inputs["v"].astype(np.float32)
    init_h_W = inputs["init_h_W"].astype(np.float32)
    init_h_b = inputs["init_h_b"].astype(np.float32)
    W_ih = inputs["W_ih"].astype(np.float32)
    b_ih = inputs["b_ih"].astype(np.float32)
    W_hh = inputs["W_hh"].astype(np.float32)
    b_hh = inputs["b_hh"].astype(np.float32)
    fc1_W = inputs["fc1_W"].astype(np.float32)
    fc1_b = inputs["fc1_b"].astype(np.float32)
    fc2_W = inputs["fc2_W"].astype(np.float32)
    fc2_b = inputs["fc2_b"].astype(np.float32)

    # host precompute: embedding projected through W_ih (emb part) + rz biases;
    # W_hh_n/b_hh_n halved for the tanh-form sigmoid r-gate; init_h_W absorbs
    # the 1/T mean divisor.
    bias_row = np.concatenate([(b_ih + b_hh)[:2 * DEC], b_ih[2 * DEC:]])
    embWb = np.concatenate([emb @ W_ih[:EMB], bias_row[None, :]], axis=0)

    bfc = lambda a: np.ascontiguousarray(a, dtype=_BF)
    shared = {
        "w_dec": bfc(W_dec),
        "w_enc": bfc(W_enc),
        "v": bfc(v.reshape(ATT, 1)),
        "embWb": bfc(embWb),
        "w_ih_c": bfc(W_ih[EMB:]),
        "w_hh_rz": bfc(W_hh[:, :2 * DEC]),
        "w_hh_n": bfc(0.5 * W_hh[:, 2 * DEC:]),
        "b_hh_n": bfc(0.5 * b_hh[2 * DEC:].reshape(1, DEC)),
        "fc1_w_h": bfc(fc1_W[:DEC]),
        "fc1_w_c": bfc(fc1_W[DEC:]),
        "fc1_b": bfc(fc1_b.reshape(1, DEC)),
        "fc2_w": bfc(fc2_W),
        "fc2_b": bfc(fc2_b.reshape(1, NCLS)),
        "init_h_w": bfc(init_h_W / T),
        "init_h_b": np.ascontiguousarray(init_h_b.reshape(DEC, 1), dtype=np.float32),
    }
    _F8 = ml_dtypes.float8_e4m3
    shared["v8"] = np.ascontiguousarray(32.0 * v.reshape(ATT, 1), dtype=_F8)
    enc_bf = enc.astype(_BF)
    in_maps = []
    for i in range(N_CORES):
        m = dict(shared)
        sh = enc_bf[i * BL:(i + 1) * BL]  # [256, T, ENC]
        tm = np.ascontiguousarray(
            sh.reshape(2, 128, T, ENC).transpose(0, 2, 1, 3))  # [2, T, 128, ENC]
        m["enc"] = tm
        m["enc8"] = np.ascontiguousarray(tm, dtype=_F8)
        in_maps.append(m)

    res = run_bass_kernel_spmd(nc, in_maps, core_ids=list(range(N_CORES)),
                               trace=bool(int(os.environ.get("KTRACE", "0"))))
    kernel.last_results = res.results
    out = np.concatenate([res.results[i]["out"] for i in range(N_CORES)], axis=0)
    if bool(int(os.environ.get("KTRACE", "0"))):
        kernel.last_exec_time_ns = res.exec_time_ns
        kernel.last_profile = res.profile_json
    return out.astype(np.float32)


# revision 27
# speedup vs baseline: 1.3816x; 1.0390x over previous
"""Trainium2 Bass kernel for nn_AttentionDecoder (Bahdanau attention + GRU greedy decoder).

Sharding: pure data parallel, B=2048 split as 256 rows per core across 8 cores.
All compute in bf16 with f32 PSUM accumulation (verified: rel err ~6e-3, no argmax flips).

Layout scheme (per core, BL=256):
  - partitions packed as p = 32*(b%4) + t  ("bd layout") so block-diagonal attention
    matmuls pack 4 batch rows per matmul; free index g = b//4 makes everything b-major.
  - attention + fc1 feature-major (feature on partitions), GRU gates batch-major,
    bridged by PE transposes.
  - context and energy matmuls run "flipped": enc/att chunks are the stationary
    operand (M=128, dense base-0 PSUM output), attn_bd/v the moving operand.
  - sigmoid computed as (1+tanh(x/2))/2 so tanh/exp/relu/copy share one ACT
    table set (no per-step ACT_TABLE_LOAD).
"""

import os
import threading
import numpy as np
import ml_dtypes

N_CORES = 8
B, T, ENC = 2048, 32, 512
DEC, ATT, EMB, NCLS, L = 256, 256, 64, 37, 10
BL = B // N_CORES  # 256 per core

_BF = ml_dtypes.bfloat16

_lock = threading.Lock()
_cache = {}


def _build():
    import concourse.bass as bass
    import concourse.tile as tile
    from concourse import bacc, mybir

    bf = mybir.dt.bfloat16
    f32 = mybir.dt.float32

    nc = bacc.Bacc("TRN2", target_bir_lowering=False, debug=False,
                   num_devices=N_CORES)

    # ---------------- DRAM parameters ----------------
    d_enc = nc.dram_tensor("enc", [2, T, 128, ENC], bf, kind="ExternalInput").ap()
    d_wdec = nc.dram_tensor("w_dec", [DEC, ATT], bf, kind="ExternalInput").ap()
    d_wenc = nc.dram_tensor("w_enc", [ENC, ATT], bf, kind="ExternalInput").ap()
    d_v = nc.dram_tensor("v", [ATT, 1], bf, kind="ExternalInput").ap()
    d_embWb = nc.dram_tensor("embWb", [NCLS + 1, 3 * DEC], bf, kind="ExternalInput").ap()
    d_wihc = nc.dram_tensor("w_ih_c", [ENC, 3 * DEC], bf, kind="ExternalInput").ap()
    d_whhrz = nc.dram_tensor("w_hh_rz", [DEC, 2 * DEC], bf, kind="ExternalInput").ap()
    d_whhn = nc.dram_tensor("w_hh_n", [DEC, DEC], bf, kind="ExternalInput").ap()
    d_bhhn = nc.dram_tensor("b_hh_n", [1, DEC], bf, kind="ExternalInput").ap()
    d_fc1h = nc.dram_tensor("fc1_w_h", [DEC, DEC], bf, kind="ExternalInput").ap()
    d_fc1c = nc.dram_tensor("fc1_w_c", [ENC, DEC], bf, kind="ExternalInput").ap()
    d_fc1b = nc.dram_tensor("fc1_b", [DEC, 1], f32, kind="ExternalInput").ap()
    d_fc2w = nc.dram_tensor("fc2_w", [DEC, NCLS], bf, kind="ExternalInput").ap()
    d_fc2b = nc.dram_tensor("fc2_b", [1, NCLS], bf, kind="ExternalInput").ap()
    d_ihw = nc.dram_tensor("init_h_w", [ENC, DEC], bf, kind="ExternalInput").ap()
    d_ihb = nc.dram_tensor("init_h_b", [DEC, 1], f32, kind="ExternalInput").ap()
    d_out = nc.dram_tensor("out", [BL, L, NCLS], f32, kind="ExternalOutput").ap()

    # constants baked into the NEFF
    ident_np = np.eye(128, dtype=_BF)
    d_ident = nc.inline_tensor(ident_np, name="ident").ap()
    onesbd_np = np.zeros((128, 4), dtype=_BF)
    for bs in range(4):
        onesbd_np[32 * bs:32 * bs + 32, bs] = 1.0 / 32.0
    d_onesbd = nc.inline_tensor(onesbd_np, name="onesbd").ap()
    d_onesrow = nc.inline_tensor(np.ones((1, 256), dtype=_BF), name="onesrow").ap()

    AluOp = mybir.AluOpType
    ActF = mybir.ActivationFunctionType

    with tile.TileContext(nc) as tc:
        with (
            tc.tile_pool(name="persist", bufs=1) as P,
            tc.tile_pool(name="wpool", bufs=1) as W,
            tc.tile_pool(name="trans", bufs=3) as TR,
            tc.tile_pool(name="small", bufs=2) as SM,
            tc.tile_pool(name="ps", bufs=3, space="PSUM") as PS,
        ):
            # ---------------- persistent SBUF tensors ----------------
            enc_bd = P.tile([128, 64, ENC], bf, tag="enc_bd")       # 64KB/part
            ep = P.tile([128, 2, T, 256], bf, tag="ep")             # enc_proj^T, t-major
            att = P.tile([128, 2, T, 256], bf, tag="att")           # tanh buffer
            hT = P.tile([128, 2, BL], bf, tag="hT")                 # h feature-major
            h_b = P.tile([128, 2, DEC], bf, tag="h_b")              # h batch-major
            ctxT = P.tile([128, 4, BL], bf, tag="ctxT")             # context feature-major
            onehotT = P.tile([NCLS + 1, BL], bf, tag="onehotT")
            out_sb = P.tile([128, 2, L, NCLS], f32, tag="out_sb")

            # ---------------- weights to SBUF ----------------
            def wload(tag, shape, src, rearr=None):
                t = W.tile(shape, bf, tag=tag)
                nc.scalar.dma_start(t[:], src if rearr is None else src.rearrange(rearr, p=128))
                return t

            w_dec = wload("w_dec", [128, 2, ATT], d_wdec, "(k p) n -> p k n")
            w_enc = wload("w_enc", [128, 4, ATT], d_wenc, "(k p) n -> p k n")
            v_sb = wload("v_sb", [128, 2, 1], d_v, "(k p) n -> p k n")
            embWb = wload("embWb", [NCLS + 1, 3 * DEC], d_embWb)
            w_ihc = wload("w_ihc", [128, 4, 3 * DEC], d_wihc, "(k p) n -> p k n")
            w_hhrz = wload("w_hhrz", [128, 2, 2 * DEC], d_whhrz, "(k p) n -> p k n")
            w_hhn = wload("w_hhn", [128, 2, DEC], d_whhn, "(k p) n -> p k n")
            bhhn = wload("bhhn", [1, DEC], d_bhhn)
            fc1h = wload("fc1h", [128, 2, DEC], d_fc1h, "(k p) n -> p k n")
            fc1c = wload("fc1c", [128, 4, DEC], d_fc1c, "(k p) n -> p k n")
            fc2w = wload("fc2w", [128, 2, NCLS], d_fc2w, "(k p) n -> p k n")
            fc2b = wload("fc2b", [1, NCLS], d_fc2b)
            ihw = wload("ihw", [128, 4, DEC], d_ihw, "(k p) n -> p k n")
            ident = wload("ident", [128, 128], d_ident)
            ihb = W.tile([128, 2, 1], f32)
            nc.scalar.dma_start(ihb[:], d_ihb.rearrange("(k p) n -> p k n", p=128))
            fc1b = W.tile([128, 2, 1], f32)
            nc.scalar.dma_start(fc1b[:], d_fc1b.rearrange("(k p) n -> p k n", p=128))
            ones1 = W.tile([1, 128], bf)
            nc.scalar.dma_start(ones1[:], d_onesrow[:, 0:128])

            meanT = TR.tile([128, 4, BL], bf, tag="meanT", bufs=1)
            # ---------------- prologue: enc_proj via xbar DMA transposes ----------------
            # encT (e-major) built by dma_start_transpose per (bt-half, e-block)
            # on the otherwise-idle DMA engines; ep matmuls consume each half.
            for bth in range(2):
                d_encb = d_enc[bth].rearrange("t b e -> (t b) e")
                encTh = TR.tile([128, 4, 4096], bf, tag="encTh", bufs=1)
                for eb in range(4):
                    nc.sync.dma_start_transpose(
                        encTh[:, eb, :],
                        d_encb[:, 128 * eb:128 * eb + 128])
                # encTh cols are now (t, b) t-major: mean reduces the outer t
                # via a strided inner view; ep copies become contiguous slabs
                for eb in range(4):
                    mr = TR.tile([128, 4, 128], f32, tag="mr", bufs=1)
                    nc.vector.tensor_reduce(
                        mr[:, eb, :],
                        encTh[:, eb, :].rearrange("p (t b) -> p b t", b=128),
                        axis=mybir.AxisListType.X, op=AluOp.add)
                    nc.vector.tensor_scalar(
                        meanT[:, eb, 128 * bth:128 * bth + 128], mr[:, eb, :],
                        1.0 / 32.0, None, op0=AluOp.mult)
                for c in range(8):
                    for ab in range(2):
                        pp = PS.tile([128, 512], f32, tag="a")
                        for eb in range(4):
                            nc.tensor.matmul(
                                pp[:],
                                w_enc[:, eb, 128 * ab:128 * ab + 128],
                                encTh[:, eb, 512 * c:512 * c + 512],
                                start=(eb == 0), stop=(eb == 3),
                            )
                        dst = ep[:, ab, 4 * c:4 * c + 4, 128 * bth:128 * bth + 128]
                        src2 = pp[:].rearrange("p (t b) -> p t b", b=128)
                        if ab == 0:
                            nc.vector.tensor_copy(dst, src2)
                        else:
                            nc.scalar.copy(dst, src2)

            # ---------------- h0 ----------------
            for db in range(2):
                hp = PS.tile([128, BL], f32, tag="a")
                for eb in range(4):
                    nc.tensor.matmul(hp[:], ihw[:, eb, 128 * db:128 * db + 128],
                                     meanT[:, eb, :], start=(eb == 0), stop=(eb == 3))
                nc.scalar.activation(hT[:, db, :], hp[:], ActF.Tanh, bias=ihb[:, db, :])
            for half in range(2):
                for db in range(2):
                    tp = PS.tile([128, 128], bf, tag="b", bufs=3)
                    nc.tensor.transpose(tp[:], hT[:, db, 128 * half:128 * half + 128],
                                        ident[:])
                    nc.vector.tensor_copy(h_b[:, half, 128 * db:128 * db + 128], tp[:])

            # enc DMA into bd layout (emitted late: only needed from step-0 ctx)
            # enc_bd[32*bs+t, g, e] = enc[4g+bs, t, e]
            for bs in range(4):
                for bth in range(2):
                    nc.scalar.dma_start(
                        enc_bd[32 * bs:32 * bs + 32, 32 * bth:32 * bth + 32, :],
                        d_enc[bth, :, bs::4, :],
                    )

            # onehotT init: y0 = 0 -> row 0 ones; row 37 = bias row (always 1)
            nc.vector.memset(onehotT[0:NCLS, :], 0)
            nc.vector.memset(onehotT[0:1, :], 1.0)
            nc.scalar.dma_start(onehotT[NCLS:NCLS + 1, :], d_onesrow[:])

            # ---------------- decode loop ----------------
            # emitted per b-half: the two halves form independent dependency
            # chains within a step, so Tile overlaps half-1 elementwise
            # (DVE/ACT) with half-0 matmuls (PE) and vice versa.
            for step in range(L):
                decT = SM.tile([128, 2, BL], bf, tag="decT")
                n_sb = SM.tile([128, 2, DEC], bf, tag="n_sb")
                tz_sb = SM.tile([128, 2, DEC], bf, tag="tz_sb")
                hidT = SM.tile([128, 2, BL], bf, tag="hidT")
                attnT = SM.tile([32, BL], bf, tag="attnT")
                attn_bd = SM.tile([128, 64, 4], bf, tag="attn_bd")
                nc.vector.memset(attn_bd[:], 0)
                for ab in range(2):
                    dp = PS.tile([128, BL], f32, tag="a")
                    for db in range(2):
                        nc.tensor.matmul(dp[:], w_dec[:, db, 128 * ab:128 * ab + 128],
                                         hT[:, db, :], start=(db == 0), stop=(db == 1))
                    nc.scalar.copy(decT[:, ab, :], dp[:])
                for half in range(2):
                    hs, he = 128 * half, 128 * half + 128
                    # s = ep + dec (broadcast over t); tanh in place
                    for ab in range(2):
                        bcast = decT[:, ab, hs:he].rearrange(
                            "p (o b) -> p o b", o=1).broadcast_to([128, T, 128])
                        nc.vector.tensor_tensor(att[:, ab, :, hs:he],
                                                ep[:, ab, :, hs:he], bcast,
                                                op=AluOp.add)
                        nc.scalar.activation(att[:, ab, :, hs:he],
                                             att[:, ab, :, hs:he], ActF.Tanh)
                    # energy (batch-major) via flipped vdot
                    ebp = PS.tile([128, T], f32, tag="b", bufs=3)
                    for t in range(T):
                        for ab in range(2):
                            nc.tensor.matmul(ebp[:, t:t + 1], att[:, ab, t, hs:he],
                                             v_sb[:, ab, :],
                                             start=(ab == 0), stop=(ab == 1))
                    expB = SM.tile([128, T], bf, tag="expB")
                    nc.scalar.activation(expB[:], ebp[:], ActF.Exp)
                    zc = SM.tile([128, 1], f32, tag="zc")
                    nc.vector.tensor_reduce(zc[:], expB[:], axis=mybir.AxisListType.X,
                                            op=AluOp.add)
                    rz = SM.tile([128, 1], f32, tag="rz")
                    nc.vector.reciprocal(rz[:], zc[:])
                    attnB = SM.tile([128, T], bf, tag="attnB")
                    nc.vector.tensor_scalar(attnB[:], expB[:], rz[:], None,
                                            op0=AluOp.mult)
                    tp = PS.tile([32, 128], bf, tag="b", bufs=3)
                    nc.tensor.transpose(tp[:], attnB[:], ident[:])
                    nc.vector.tensor_copy(attnT[:, hs:he], tp[:])
                    # attn blockdiag build for this half
                    for bs in range(4):
                        nc.vector.tensor_copy(
                            attn_bd[32 * bs:32 * bs + 32,
                                    32 * half:32 * half + 32, bs],
                            attnT[:, hs + bs:he:4],
                        )
                    # context feature-major (flipped blockdiag)
                    for eb in range(4):
                        cp = PS.tile([128, 128], f32, tag="cp", bufs=2)
                        for gr in range(32):
                            g = 32 * half + gr
                            nc.tensor.matmul(cp[:, 4 * gr:4 * gr + 4],
                                             enc_bd[:, g, 128 * eb:128 * eb + 128],
                                             attn_bd[:, g, :], start=True, stop=True)
                        nc.vector.tensor_copy(ctxT[:, eb, hs:he], cp[:])
                    # GRU gates
                    gi_rz = PS.tile([128, 2 * DEC], f32, tag="a")
                    gi_n = PS.tile([128, DEC], f32, tag="b", bufs=3)
                    ghn = PS.tile([128, DEC], f32, tag="b", bufs=3)
                    oh = onehotT[:, hs:he]
                    nc.tensor.matmul(gi_rz[:], oh, embWb[:, 0:512],
                                     start=True, stop=False)
                    nc.tensor.matmul(gi_n[:], oh, embWb[:, 512:768],
                                     start=True, stop=False)
                    for eb in range(4):
                        ct = ctxT[:, eb, hs:he]
                        nc.tensor.matmul(gi_rz[:], ct, w_ihc[:, eb, 0:512],
                                         start=False, stop=False)
                        nc.tensor.matmul(gi_n[:], ct, w_ihc[:, eb, 512:768],
                                         start=False, stop=False)
                    nc.tensor.matmul(ghn[:], ones1[:], bhhn[:], start=True, stop=False)
                    nc.tensor.matmul(gi_n[:], ones1[:], bhhn[:], start=False, stop=False)
                    for db in range(2):
                        hTs = hT[:, db, hs:he]
                        nc.tensor.matmul(gi_rz[:], hTs, w_hhrz[:, db, :], start=False,
                                         stop=(db == 1))
                        nc.tensor.matmul(ghn[:], hTs, w_hhn[:, db, :], start=False,
                                         stop=(db == 1))
                        nc.tensor.matmul(gi_n[:], hTs, w_hhn[:, db, :], start=False,
                                         stop=(db == 1))
                    # r-gate via tanh: npre = gi_n + tanh(rx/2)*ghn2
                    tr_sb = SM.tile([128, DEC], bf, tag="tr_sb")
                    nc.scalar.activation(tr_sb[:], gi_rz[:, 0:DEC], ActF.Tanh, scale=0.5)
                    nc.scalar.activation(tz_sb[:, half, :], gi_rz[:, DEC:2 * DEC],
                                         ActF.Tanh, scale=0.5)
                    rhn = SM.tile([128, DEC], bf, tag="rhn")
                    nc.vector.tensor_tensor(rhn[:], tr_sb[:], ghn[:], op=AluOp.mult)
                    npre = SM.tile([128, DEC], bf, tag="npre")
                    nc.vector.tensor_tensor(npre[:], gi_n[:], rhn[:], op=AluOp.add)
                    nc.scalar.activation(n_sb[:, half, :], npre[:], ActF.Tanh)
                    # h_new = 0.5*(t1 + tz*t1) + n,  t1 = h - n
                    t1 = SM.tile([128, DEC], bf, tag="t1")
                    nc.vector.tensor_tensor(t1[:], h_b[:, half, :], n_sb[:, half, :],
                                            op=AluOp.subtract)
                    t2 = SM.tile([128, DEC], bf, tag="t2")
                    nc.vector.tensor_tensor(t2[:], tz_sb[:, half, :], t1[:],
                                            op=AluOp.mult)
                    t3 = SM.tile([128, DEC], bf, tag="t3")
                    nc.vector.tensor_tensor(t3[:], t1[:], t2[:], op=AluOp.add)
                    nc.vector.scalar_tensor_tensor(h_b[:, half, :], t3[:], 0.5,
                                                   n_sb[:, half, :],
                                                   op0=AluOp.mult, op1=AluOp.add)
                    for db in range(2):
                        tp = PS.tile([128, 128], bf, tag="b", bufs=3)
                        nc.tensor.transpose(tp[:],
                                            h_b[:, half, 128 * db:128 * db + 128],
                                            ident[:])
                        nc.vector.tensor_copy(hT[:, db, hs:he], tp[:])
                    # fc1 feature-major, full-b (only once, after both halves)
                    if half == 1:
                        for db in range(2):
                            fp = PS.tile([128, BL], f32, tag="a")
                            for k in range(2):
                                nc.tensor.matmul(fp[:],
                                                 fc1h[:, k, 128 * db:128 * db + 128],
                                                 hT[:, k, :], start=(k == 0), stop=False)
                            for eb in range(4):
                                nc.tensor.matmul(fp[:],
                                                 fc1c[:, eb, 128 * db:128 * db + 128],
                                                 ctxT[:, eb, :], start=False,
                                                 stop=(eb == 3))
                            nc.scalar.activation(hidT[:, db, :], fp[:], ActF.Relu,
                                                 bias=fc1b[:, db, :])
                    if half == 1:
                        for h2 in range(2):
                            h2s, h2e = 128 * h2, 128 * h2 + 128
                            lp = PS.tile([128, NCLS], f32, tag="b", bufs=3)
                            nc.tensor.matmul(lp[:], ones1[:], fc2b[:],
                                             start=True, stop=False)
                            for db in range(2):
                                nc.tensor.matmul(lp[:], hidT[:, db, h2s:h2e],
                                                 fc2w[:, db, :], start=False,
                                                 stop=(db == 1))
                            nc.scalar.copy(out_sb[:, h2, step, :], lp[:])
                            if step < L - 1:
                                mx = SM.tile([128, 1], f32, tag="zc")
                                nc.vector.tensor_reduce(mx[:], lp[:],
                                                        axis=mybir.AxisListType.X,
                                                        op=AluOp.max)
                                ohB = SM.tile([128, NCLS], bf, tag="ohB")
                                nc.vector.tensor_tensor(
                                    ohB[:], lp[:],
                                    mx[:].broadcast_to([128, NCLS]), op=AluOp.is_equal)
                                tp = PS.tile([NCLS, 128], bf, tag="b", bufs=3)
                                nc.tensor.transpose(tp[:], ohB[:], ident[:])
                                nc.vector.tensor_copy(onehotT[0:NCLS, h2s:h2e], tp[:])

            # ---------------- output DMA ----------------
            for half in range(2):
                nc.sync.dma_start(
                    d_out[128 * half:128 * half + 128],
                    out_sb[:, half, :, :],
                )

    nc.compile()
    return nc


def _get_nc():
    with _lock:
        if "nc" not in _cache:
            _cache["nc"] = _build()
        return _cache["nc"]


def kernel(**inputs):
    nc = _get_nc()
    from concourse.bass_utils import run_bass_kernel_spmd

    enc = np.ascontiguousarray(inputs["encoder_outputs"], dtype=np.float32)
    emb = inputs["emb"].astype(np.float32)
    W_enc = inputs["W_enc"].astype(np.float32)
    W_dec = inputs["W_dec"].astype(np.float32)
    v = inputs["v"].astype(np.float32)
    init_h_W = inputs["init_h_W"].astype(np.float32)
    init_h_b = inputs["init_h_b"].astype(np.float32)
    W_ih = inputs["W_ih"].astype(np.float32)
    b_ih = inputs["b_ih"].astype(np.float32)
    W_hh = inputs["W_hh"].astype(np.float32)
    b_hh = inputs["b_hh"].astype(np.float32)
    fc1_W = inputs["fc1_W"].astype(np.float32)
    fc1_b = inputs["fc1_b"].astype(np.float32)
    fc2_W = inputs["fc2_W"].astype(np.float32)
    fc2_b = inputs["fc2_b"].astype(np.float32)

    # host precompute: embedding projected through W_ih (emb part) + rz biases;
    # W_hh_n/b_hh_n halved for the tanh-form sigmoid r-gate
    bias_row = np.concatenate([(b_ih + b_hh)[:2 * DEC], b_ih[2 * DEC:]])
    embWb = np.concatenate([emb @ W_ih[:EMB], bias_row[None, :]], axis=0)

    bfc = lambda a: np.ascontiguousarray(a, dtype=_BF)
    shared = {
        "w_dec": bfc(W_dec),
        "w_enc": bfc(W_enc),
        "v": bfc(v.reshape(ATT, 1)),
        "embWb": bfc(embWb),
        "w_ih_c": bfc(W_ih[EMB:]),
        "w_hh_rz": bfc(W_hh[:, :2 * DEC]),
        "w_hh_n": bfc(0.5 * W_hh[:, 2 * DEC:]),
        "b_hh_n": bfc(0.5 * b_hh[2 * DEC:].reshape(1, DEC)),
        "fc1_w_h": bfc(fc1_W[:DEC]),
        "fc1_w_c": bfc(fc1_W[DEC:]),
        "fc1_b": np.ascontiguousarray(fc1_b.reshape(DEC, 1), dtype=np.float32),
        "fc2_w": bfc(fc2_W),
        "fc2_b": bfc(fc2_b.reshape(1, NCLS)),
        "init_h_w": bfc(init_h_W),
        "init_h_b": np.ascontiguousarray(init_h_b.reshape(DEC, 1), dtype=np.float32),
    }
    enc_bf = enc.astype(_BF)
    in_maps = []
    for i in range(N_CORES):
        m = dict(shared)
        sh = enc_bf[i * BL:(i + 1) * BL]
        m["enc"] = np.ascontiguousarray(
            sh.reshape(2, 128, T, ENC).transpose(0, 2, 1, 3))
        in_maps.append(m)

    res = run_bass_kernel_spmd(nc, in_maps, core_ids=list(range(N_CORES)),
                               trace=bool(int(os.environ.get("KTRACE", "0"))))
    out = np.concatenate([res.results[i]["out"] for i in range(N_CORES)], axis=0)
    if bool(int(os.environ.get("KTRACE", "0"))):
        kernel.last_exec_time_ns = res.exec_time_ns
        kernel.last_profile = res.profile_json
    return out.astype(np.float32)

